# revision 13
# baseline (speedup 1.0000x reference)
"""ARTabPFN forward kernel for 8 TRN2 NeuronCores.

Sharding: 2 batch groups x 4-way row sharding (386 rows/core).
Device does: 4 transformer layers + final norm + head MLP.
Host does: embedding (tiny) and the K=5 mixture/loss epilogue (tiny).

Activations live transposed in SBUF: [feature_on_partitions, rows_on_free].
All matmuls run as float32r (full PE speed, ~1e-4 rounding).
Row attention exploits sparsity: every row attends only to the first
NC+NB=1032 columns (ctx+buf), plus a masked self-term for target rows,
so K/V are gathered per layer only for those 1032 rows (AllGather over
each 4-core group).
"""

import math
import numpy as np

import concourse.bass as bass
import concourse.bacc as bacc
import concourse.tile as tile
import concourse.mybir as mybir
from concourse.bass_utils import run_bass_kernel_spmd

F32R = mybir.dt.float32r
F32 = mybir.dt.float32
AF = mybir.ActivationFunctionType

B, NC, NB, NT, CF = 2, 1024, 8, 512, 32
D, H, L, DFF, K = 512, 8, 4, 1024, 5
R = NC + NB + NT          # 1544
NKV = NC + NB             # 1032
ROWS = R // 4             # 386 rows per core
DH = D // H               # 64
N_CORES = 8
STD_MIN = 1e-3

# peer p in a group owns rows [p*ROWS, (p+1)*ROWS); kv rows are < NKV
PEER_KV = [min(max(NKV - p * ROWS, 0), ROWS) for p in range(4)]   # [386,386,260,0]
CONTRIB_ROWS = D + ROWS   # 898: rows 0:512 = kT (cols 0:386), rows 512:898 = V row-major
KCHUNKS = [(s, min(128, NKV - s)) for s in range(0, NKV, 128)]    # 9 chunks, last = 8

_COMPILED = None


def _build():
    nc = bacc.Bacc("TRN2", target_bir_lowering=False, debug=False,
                   num_devices=N_CORES)

    def din(name, shape, dt=F32R):
        return nc.declare_dram_parameter(name, list(shape), dt, isOutput=False)

    x0 = din("x0", [D, ROWS])
    cones = din("cones", [128, 1])
    onesrow = din("onesrow", [1, 128])
    blockones = din("blockones", [128, 32])
    expander = din("expander", [8, D])
    sel8 = din("sel8", [1, 64])
    eye8 = din("eye8", [8, 8])
    vones = din("vones", [128, 8])
    bufmask = din("bufmask", [NB, ROWS])
    istgt8 = din("istgt8", [NB, ROWS])

    LW = []
    for l in range(L):
        LW.append({
            "wfvo": din(f"wfvo{l}", [D, D]), "bfvo": din(f"bfvo{l}", [D, 1], F32),
            "wq": din(f"wq{l}", [D, D]), "bq": din(f"bq{l}", [D, 1], F32),
            "wk": din(f"wk{l}", [D, D]),
            "wv": din(f"wv{l}", [D, D]),
            "wo": din(f"wo{l}", [D, D]), "bo": din(f"bo{l}", [D, 1], F32),
            "wf1": din(f"wf1_{l}", [D, DFF]), "bf1": din(f"bf1_{l}", [DFF, 1], F32),
            "wf2": din(f"wf2_{l}", [DFF, D]), "bf2": din(f"bf2_{l}", [D, 1], F32),
            "g1": din(f"g1_{l}", [D, 1], F32), "c1": din(f"c1_{l}", [D, 1], F32),
            "g2": din(f"g2_{l}", [D, 1], F32), "c2": din(f"c2_{l}", [D, 1], F32),
            "g3": din(f"g3_{l}", [D, 1], F32), "c3": din(f"c3_{l}", [D, 1], F32),
        })
    wh1 = din("wh1", [D, DFF]); bh1 = din("bh1", [DFF, 1], F32)
    wh2 = din("wh2", [DFF, 15]); bh2 = din("bh2", [15, 1], F32)
    gf = din("gf", [D, 1], F32); cf = din("cf", [D, 1], F32)

    raw_out = nc.declare_dram_parameter("raw", [15, ROWS], F32, isOutput=True)

    with tile.TileContext(nc) as tc:
        ctx_lp = nc.allow_low_precision(reason="deliberate f32r compute")
        ctx_lp.__enter__()
        with tc.tile_pool(name="const", bufs=1) as constp, \
             tc.tile_pool(name="acts", bufs=1) as actp, \
             tc.tile_pool(name="wd", bufs=2) as wdp, \
             tc.tile_pool(name="wff", bufs=1) as wffp, \
             tc.tile_pool(name="wvp", bufs=1) as wvp, \
             tc.tile_pool(name="kv", bufs=1) as kvp, \
             tc.tile_pool(name="attn", bufs=2) as atp, \
             tc.tile_pool(name="pvp", bufs=1) as pvp, \
             tc.tile_pool(name="small", bufs=1) as smp, \
             tc.tile_pool(name="pp", bufs=2, space="PSUM") as pp, \
             tc.tile_pool(name="ps", bufs=2, space="PSUM") as ps, \
             tc.tile_pool(name="pav", bufs=2, space="PSUM") as pav, \
             tc.tile_pool(name="pt", bufs=2, space="PSUM") as pt, \
             tc.tile_pool(name="dram", bufs=2, space="DRAM") as dramp:

            # ---- constants ----
            t_cones = constp.tile([128, 1], F32R, tag="cones")
            nc.sync.dma_start(t_cones[:], cones[:])
            t_onesrow = constp.tile([1, 128], F32R, tag="onesrow")
            nc.sync.dma_start(t_onesrow[:], onesrow[:])
            t_blko = constp.tile([128, 32], F32R, tag="blko")
            nc.sync.dma_start(t_blko[:], blockones[:])
            t_exp = constp.tile([8, D], F32R, tag="exp")
            nc.sync.dma_start(t_exp[:], expander[:])
            t_sel8 = constp.tile([1, 64], F32R, tag="sel8")
            nc.sync.dma_start(t_sel8[:], sel8[:])
            t_eye8 = constp.tile([8, 8], F32R, tag="eye8")
            nc.sync.dma_start(t_eye8[:], eye8[:])
            t_bufm = constp.tile([NB, ROWS], F32R, tag="bufm")
            nc.sync.dma_start(t_bufm[:], bufmask[:])
            t_istgt = constp.tile([NB, ROWS], F32R, tag="istgt")
            nc.sync.dma_start(t_istgt[:], istgt8[:])

            def load_w(drh, dout, pool, tag):
                """Load [D_in, dout] weights as tiles of [128, dout]."""
                nin = drh.shape[0]
                ts = []
                for ci in range(nin // 128):
                    t = pool.tile([128, dout], F32R, tag=f"{tag}{ci}")
                    nc.sync.dma_start(t[:], drh[ci * 128:(ci + 1) * 128, :])
                    ts.append(t)
                return ts

            def load_b(drh, tag, n=D):
                ts = []
                for ci in range(n // 128):
                    t = smp.tile([128, 1], F32, tag=f"{tag}{ci}")
                    nc.sync.dma_start(t[:], drh[ci * 128:(ci + 1) * 128, :])
                    ts.append(t)
                return ts

            def proj_t(xin, wts, epi, dout):
                """Transposed projection: out[co][128, ROWS] = sum_ci W[ci][:,co].T @ xin[ci].
                epi(psum, co) -> sbuf tile."""
                outs = []
                for co in range(dout // 128):
                    psum = ps.tile([128, ROWS], F32, tag="ps")
                    nci = len(xin)
                    for ci in range(nci):
                        nc.tensor.matmul(psum[:], wts[ci][:, co * 128:(co + 1) * 128],
                                         xin[ci][:], start=(ci == 0), stop=(ci == nci - 1))
                    outs.append(epi(psum, co))
                return outs

            def layernorm(xin, gts, cts, tag):
                """LN over features (partitions). xin: 4 tiles [128, ROWS] f32r."""
                pst = pt.tile([15, ROWS], F32, tag="pt")
                for ci in range(4):
                    nc.tensor.matmul(pst[0:1, :], t_cones[:], xin[ci][:],
                                     start=(ci == 0), stop=(ci == 3))
                mu = smp.tile([1, ROWS], F32, tag="mu")
                nc.scalar.activation(mu[:], pst[0:1, :], AF.Copy, scale=1.0 / D)
                psq = pt.tile([15, ROWS], F32, tag="pt")
                for ci in range(4):
                    sq = actp.tile([128, ROWS], F32R, tag="lnsq")
                    nc.vector.tensor_mul(sq[:], xin[ci][:], xin[ci][:])
                    nc.tensor.matmul(psq[0:1, :], t_cones[:], sq[:],
                                     start=(ci == 0), stop=(ci == 3))
                exq = smp.tile([1, ROWS], F32, tag="exq")
                nc.scalar.activation(exq[:], psq[0:1, :], AF.Copy, scale=1.0 / D)
                var = smp.tile([1, ROWS], F32, tag="var")
                nc.vector.tensor_mul(var[:], mu[:], mu[:])
                nc.vector.tensor_sub(var[:], exq[:], var[:])
                nc.vector.tensor_scalar_add(var[:], var[:], 1e-5)
                nc.scalar.activation(exq[:], var[:], AF.Sqrt)
                rr = smp.tile([1, ROWS], F32R, tag="rr")
                nc.vector.reciprocal(rr[:], exq[:])
                aa = smp.tile([1, ROWS], F32R, tag="aa")
                nc.vector.tensor_mul(aa[:], mu[:], rr[:])
                nc.vector.tensor_scalar_mul(aa[:], aa[:], -1.0)
                # broadcast r, a across partitions via K=1 matmul
                pbs = ps.tile([128, ROWS], F32, tag="ps")
                nc.tensor.matmul(pbs[:], t_onesrow[:], rr[:], start=True, stop=True)
                pba = ps.tile([128, ROWS], F32, tag="ps")
                nc.tensor.matmul(pba[:], t_onesrow[:], aa[:], start=True, stop=True)
                outs = []
                for ci in range(4):
                    t = actp.tile([128, ROWS], F32, tag="lnt")
                    nc.vector.tensor_mul(t[:], xin[ci][:], pbs[:])
                    nc.vector.tensor_add(t[:], t[:], pba[:])
                    o = actp.tile([128, ROWS], F32R, tag=f"{tag}{ci}")
                    nc.scalar.activation(o[:], t[:], AF.Identity,
                                         bias=cts[ci][:], scale=gts[ci][:])
                    outs.append(o)
                return outs

            # ---- initial activations ----
            x = []
            for ci in range(4):
                t = actp.tile([128, ROWS], F32R, tag=f"x{ci}")
                nc.sync.dma_start(t[:], x0[ci * 128:(ci + 1) * 128, :])
                x.append(t)

            for l in range(L):
                lw = LW[l]
                w_fvo = load_w(lw["wfvo"], D, wdp, "wd")
                b_fvo = load_b(lw["bfvo"], "bfvo")
                g1 = load_b(lw["g1"], "g1"); c1 = load_b(lw["c1"], "c1")

                # ---- feature attention (collapsed) + residual + LN1 ----
                def epi_fattn(psum, co):
                    t = actp.tile([128, ROWS], F32R, tag=f"res{co}")
                    nc.scalar.activation(t[:], psum[:], AF.Identity, bias=b_fvo[co][:])
                    nc.vector.tensor_add(t[:], t[:], x[co][:])
                    return t
                r1 = proj_t(x, w_fvo, epi_fattn, D)
                xf = layernorm(r1, g1, c1, "xf")

                # ---- K (no bias: cancels in softmax) and V row-major (bias folded into o-proj) ----
                w_k = load_w(lw["wk"], D, wdp, "wd")

                def epi_plain(tag):
                    def f(psum, co):
                        t = actp.tile([128, ROWS], F32R, tag=f"{tag}{co}")
                        nc.vector.tensor_copy(t[:], psum[:])
                        return t
                    return f
                kT = proj_t(xf, w_k, epi_plain("kT"), D)

                w_v = load_w(lw["wv"], D, wvp, "wv")
                contrib = dramp.tile([CONTRIB_ROWS, D], F32R, tag="contrib")
                gathered = dramp.tile([4 * CONTRIB_ROWS, D], F32R, tag="gathered")
                for ci in range(4):
                    nc.sync.dma_start(contrib[ci * 128:(ci + 1) * 128, 0:ROWS], kT[ci][:])
                # V row-major: out[row_chunk, 512]
                vrm = []
                for rt in range(4):
                    rc = min(128, ROWS - rt * 128)
                    psum = pp.tile([128, D], F32, tag="pp")
                    for ci in range(4):
                        nc.tensor.matmul(psum[0:rc, :],
                                         xf[ci][:, rt * 128:rt * 128 + rc],
                                         w_v[ci][:], start=(ci == 0), stop=(ci == 3))
                    t = actp.tile([128, D], F32R, tag=f"vrm{rt}")
                    nc.vector.tensor_copy(t[0:rc, :], psum[0:rc, :])
                    vrm.append((t, rc))
                    nc.sync.dma_start(
                        contrib[D + rt * 128:D + rt * 128 + rc, :], t[0:rc, :])

                nc.gpsimd.collective_compute(
                    "AllGather", mybir.AluOpType.bypass,
                    ins=[contrib.opt()], outs=[gathered.opt()],
                    replica_groups=[[0, 1, 2, 3], [4, 5, 6, 7]])

                # ---- Q (with bias), vT (transposed V for self term) while gather flies ----
                w_q = load_w(lw["wq"], D, wdp, "wd")
                b_q = load_b(lw["bq"], "bq")

                def epi_q(psum, co):
                    t = actp.tile([128, ROWS], F32R, tag=f"qT{co}")
                    nc.scalar.activation(t[:], psum[:], AF.Identity, bias=b_q[co][:])
                    return t
                qT = proj_t(xf, w_q, epi_q, D)
                vT = proj_t(xf, w_v, epi_plain("vT"), D)

                # self scores: s_self[h, row] = sum_dh qT*kT
                pss = pt.tile([15, ROWS], F32, tag="pt")
                for ci in range(4):
                    t = actp.tile([128, ROWS], F32R, tag="qk")
                    nc.vector.tensor_mul(t[:], qT[ci][:], kT[ci][:])
                    nc.tensor.matmul(pss[0:8, :], t_blko[:, ci * 8:(ci + 1) * 8],
                                     t[:], start=(ci == 0), stop=(ci == 3))
                sst = smp.tile([15, ROWS], F32, tag="rawt")
                nc.vector.tensor_copy(sst[0:8, :], pss[0:8, :])
                pself = smp.tile([8, ROWS], F32R, tag="pself")
                nc.scalar.activation(pself[:], sst[0:8, :], AF.Exp, scale=1.0 / math.sqrt(DH))
                nc.vector.tensor_mul(pself[:], pself[:], t_istgt[:])

                # ---- repack gathered K/V ----
                kg = []
                for ci in range(4):
                    t = kvp.tile([128, NKV], F32R, tag=f"kg{ci}")
                    off = 0
                    for p in range(4):
                        n = PEER_KV[p]
                        if n == 0:
                            continue
                        nc.sync.dma_start(
                            t[:, off:off + n],
                            gathered[p * CONTRIB_ROWS + ci * 128:
                                     p * CONTRIB_ROWS + (ci + 1) * 128, 0:n])
                        off += n
                    kg.append(t)
                vg = []
                for (s, n) in KCHUNKS:
                    t = kvp.tile([128, 8, DH + 1], F32R, tag=f"vg{s}")
                    nc.sync.dma_start(t[0:n, :, DH:DH + 1], vones[0:n, :])
                    # rows s..s+n map onto peer blocks
                    r0 = s
                    doff = 0
                    while r0 < s + n:
                        p = min(r0 // ROWS, 2)
                        lr0 = r0 - p * ROWS
                        cnt = min(PEER_KV[p] - lr0, s + n - r0)
                        src = gathered[p * CONTRIB_ROWS + D + lr0:
                                       p * CONTRIB_ROWS + D + lr0 + cnt, :]
                        nc.sync.dma_start(
                            t[doff:doff + cnt, :, 0:DH],
                            src.rearrange("p (h d) -> p h d", h=8))
                        doff += cnt
                        r0 += cnt
                    vg.append((t, n))

                # ---- attention per head ----
                pvs = []
                dns = []
                for h in range(H):
                    ci, off = h // 2, (h % 2) * 64
                    pchunks = []
                    for ki, (s, n) in enumerate(KCHUNKS):
                        psc = ps.tile([128, ROWS], F32, tag="ps")
                        nc.tensor.matmul(psc[0:n, :],
                                         kg[ci][off:off + 64, s:s + n],
                                         qT[ci][off:off + 64, :],
                                         start=True, stop=True)
                        pe = atp.tile([128, ROWS], F32R, tag="pch")
                        nc.scalar.activation(pe[0:n, :], psc[0:n, :], AF.Exp,
                                             scale=1.0 / math.sqrt(DH))
                        if s == NC:  # buffer columns: causal/visibility mask
                            nc.vector.tensor_mul(pe[0:n, :], pe[0:n, :], t_bufm[:])
                        pchunks.append((pe, n))
                    pavt = pav.tile([DH + 1, ROWS], F32, tag="pav")
                    for ki, ((vt, n), (pe, n2)) in enumerate(zip(vg, pchunks)):
                        nc.tensor.matmul(pavt[:], vt[0:n, h, :], pe[0:n, :],
                                         start=(ki == 0), stop=(ki == len(vg) - 1))
                    pv = pvp.tile([128, ROWS], F32, tag=f"pv{h}")
                    nc.vector.tensor_copy(pv[(h % 2) * 64:(h % 2) * 64 + 64, :],
                                          pavt[0:DH, :])
                    dn = pvp.tile([1, ROWS], F32R, tag=f"dn{h}")
                    nc.vector.tensor_copy(dn[:], pavt[DH:DH + 1, :])
                    pvs.append(pv)
                    dns.append(dn)

                # denominators: assemble [8, ROWS] in psum, add self term
                pden = pt.tile([15, ROWS], F32, tag="pt")
                for h in range(H):
                    nc.tensor.matmul(pden[0:8, :], t_sel8[:, h * 8:(h + 1) * 8],
                                     dns[h][:], start=(h == 0), stop=False)
                nc.tensor.matmul(pden[0:8, :], t_eye8[:], pself[:],
                                 start=False, stop=True)
                rec = smp.tile([8, ROWS], F32R, tag="rec")
                nc.vector.reciprocal(rec[:], pden[0:8, :])
                selfw = smp.tile([8, ROWS], F32R, tag="selfw")
                nc.vector.tensor_mul(selfw[:], pself[:], rec[:])

                attn = []
                for ci in range(4):
                    prb = ps.tile([128, ROWS], F32, tag="ps")
                    nc.tensor.matmul(prb[:], t_exp[:, ci * 128:(ci + 1) * 128],
                                     rec[:], start=True, stop=True)
                    psw = ps.tile([128, ROWS], F32, tag="ps")
                    nc.tensor.matmul(psw[:], t_exp[:, ci * 128:(ci + 1) * 128],
                                     selfw[:], start=True, stop=True)
                    t = actp.tile([128, ROWS], F32R, tag=f"attn{ci}")
                    nc.vector.tensor_mul(t[:], vT[ci][:], psw[:])
                    t2 = actp.tile([128, ROWS], F32, tag="t2")
                    for hh in range(2):
                        o = hh * 64
                        nc.vector.tensor_mul(t2[o:o + 64, :],
                                             pvs[2 * ci + hh][o:o + 64, :],
                                             prb[o:o + 64, :])
                        nc.vector.tensor_add(t[o:o + 64, :],
                                             t[o:o + 64, :], t2[o:o + 64, :])
                    attn.append(t)

                # ---- o-proj + residual + LN2 ----
                w_o = load_w(lw["wo"], D, wdp, "wd")
                b_o = load_b(lw["bo"], "bo")
                g2 = load_b(lw["g2"], "g2"); c2 = load_b(lw["c2"], "c2")

                def epi_o(psum, co):
                    t = actp.tile([128, ROWS], F32R, tag=f"res{co}")
                    nc.scalar.activation(t[:], psum[:], AF.Identity, bias=b_o[co][:])
                    nc.vector.tensor_add(t[:], t[:], xf[co][:])
                    return t
                r2 = proj_t(attn, w_o, epi_o, D)
                x2 = layernorm(r2, g2, c2, "x2")

                # ---- MLP + residual + LN3 ----
                w_f1 = load_w(lw["wf1"], DFF, wffp, "wf")
                b_f1 = load_b(lw["bf1"], "bf1", DFF)

                def epi_g(psum, co):
                    t = actp.tile([128, ROWS], F32R, tag=f"h1_{co}")
                    nc.scalar.activation(t[:], psum[:], AF.Gelu, bias=b_f1[co][:])
                    return t
                h1 = proj_t(x2, w_f1, epi_g, DFF)

                w_f2 = load_w(lw["wf2"], D, wffp, "wf")
                b_f2 = load_b(lw["bf2"], "bf2")
                g3 = load_b(lw["g3"], "g3"); c3 = load_b(lw["c3"], "c3")

                def epi_f2(psum, co):
                    t = actp.tile([128, ROWS], F32R, tag=f"res{co}")
                    nc.scalar.activation(t[:], psum[:], AF.Identity, bias=b_f2[co][:])
                    nc.vector.tensor_add(t[:], t[:], x2[co][:])
                    return t
                r3 = proj_t(h1, w_f2, epi_f2, D)
                x = layernorm(r3, g3, c3, "x")

            # ---- final norm + head ----
            gft = load_b(gf, "gf"); cft = load_b(cf, "cf")
            z = layernorm(x, gft, cft, "res")
            w_h1 = load_w(wh1, DFF, wffp, "wf")
            b_h1 = load_b(bh1, "bh1", DFF)

            def epi_h1(psum, co):
                t = actp.tile([128, ROWS], F32R, tag=f"h1_{co}")
                nc.scalar.activation(t[:], psum[:], AF.Gelu, bias=b_h1[co][:])
                return t
            hh = proj_t(z, w_h1, epi_h1, DFF)

            w_h2 = load_w(wh2, 15, wffp, "wf")
            b_h2t = smp.tile([15, 1], F32, tag="bh2")
            nc.sync.dma_start(b_h2t[:], bh2[:])
            ph2 = pt.tile([15, ROWS], F32, tag="pt")
            for ci in range(8):
                nc.tensor.matmul(ph2[:], w_h2[ci][:], hh[ci][:],
                                 start=(ci == 0), stop=(ci == 7))
            rawt = smp.tile([15, ROWS], F32, tag="rawt")
            nc.scalar.activation(rawt[:], ph2[:], AF.Identity, bias=b_h2t[:])
            nc.sync.dma_start(raw_out[:], rawt[:])

    nc.compile()
    return nc


def _prep_inputs(params, x_context, y_context, x_buffer, y_buffer,
                 x_target, y_target):
    g = lambda a: np.asarray(a, dtype=np.float32)

    # ---- host embedding (tiny: ~1.6M FLOPs) ----
    wx, bx = g(params["x_embed"]["W"]), g(params["x_embed"]["b"])
    wy, by = g(params["y_embed"]["W"]), g(params["y_embed"]["b"])
    marker = g(params["marker"])
    ar = g(params["ar_tokens"])
    def emb(xx, yy=None):
        e = g(xx).mean(axis=2, keepdims=True) @ wx + bx
        if yy is not None:
            e = e + g(yy)[..., None] @ wy + by
        return e
    ctx = emb(x_context, y_context) + marker[1]
    buf = emb(x_buffer, y_buffer) + marker[2] + ar
    tgt = emb(x_target) + marker[0]
    x = np.concatenate([ctx, buf, tgt], axis=1)          # [B, R, D]

    # ---- per-layer fused weights ----
    layers = []
    for lp in params["layers"]:
        wvf, bvf = g(lp["attn_f"]["v"]["W"]), g(lp["attn_f"]["v"]["b"])
        wof, bof = g(lp["attn_f"]["o"]["W"]), g(lp["attn_f"]["o"]["b"])
        wo, bo = g(lp["attn_r"]["o"]["W"]), g(lp["attn_r"]["o"]["b"])
        bv = g(lp["attn_r"]["v"]["b"])
        layers.append({
            "wfvo": wvf @ wof, "bfvo": (bvf @ wof + bof)[:, None],
            "wq": g(lp["attn_r"]["q"]["W"]), "bq": g(lp["attn_r"]["q"]["b"])[:, None],
            "wk": g(lp["attn_r"]["k"]["W"]),
            "wv": g(lp["attn_r"]["v"]["W"]),
            "wo": wo, "bo": (bv @ wo + bo)[:, None],
            "wf1": g(lp["ff1"]["W"]), "bf1": g(lp["ff1"]["b"])[:, None],
            "wf2": g(lp["ff2"]["W"]), "bf2": g(lp["ff2"]["b"])[:, None],
            "g1": g(lp["n1"]["g"])[:, None], "c1": g(lp["n1"]["b"])[:, None],
            "g2": g(lp["n2"]["g"])[:, None], "c2": g(lp["n2"]["b"])[:, None],
            "g3": g(lp["n3"]["g"])[:, None], "c3": g(lp["n3"]["b"])[:, None],
        })

    # ---- masks / constants ----
    idx = np.arange(R)
    is_tgt = (idx >= NKV).astype(np.float32)
    # buffer-column visibility (cols NC..NC+NB) per query row
    bm = np.zeros((R, NB), np.float32)
    bcol = NC + np.arange(NB)
    is_buf_row = (idx >= NC) & (idx < NKV)
    bm[is_buf_row[:, None] & (bcol[None, :] <= idx[:, None])] = 1.0
    bm[idx >= NKV, :] = 1.0

    expander = np.zeros((8, D), np.float32)
    for m in range(D):
        expander[m // DH, m] = 1.0
    blockones = np.zeros((128, 32), np.float32)
    for ci in range(4):
        blockones[0:64, ci * 8 + 2 * ci] = 1.0
        blockones[64:128, ci * 8 + 2 * ci + 1] = 1.0

    sel8 = np.zeros((1, 64), np.float32)
    for h in range(8):
        sel8[0, h * 8 + h] = 1.0
    common = {
        "sel8": sel8,
        "eye8": np.eye(8, dtype=np.float32),
        "vones": np.ones((128, 8), np.float32),
        "cones": np.ones((128, 1), np.float32),
        "onesrow": np.ones((1, 128), np.float32),
        "blockones": blockones,
        "expander": expander,
        "wh1": g(params["head1"]["W"]), "bh1": g(params["head1"]["b"])[:, None],
        "wh2": g(params["head2"]["W"]), "bh2": g(params["head2"]["b"])[:, None],
        "gf": g(params["final_norm"]["g"])[:, None],
        "cf": g(params["final_norm"]["b"])[:, None],
    }
    for l, lw in enumerate(layers):
        for k, v in lw.items():
            name = f"{k}{l}" if not k[-1].isdigit() else f"{k}_{l}"
            common[name] = np.ascontiguousarray(v)

    in_maps = []
    for core in range(N_CORES):
        b, p = core // 4, core % 4
        r0, r1 = p * ROWS, (p + 1) * ROWS
        m = dict(common)
        m["x0"] = np.ascontiguousarray(x[b, r0:r1, :].T)
        m["bufmask"] = np.ascontiguousarray(bm[r0:r1, :].T)
        m["istgt8"] = np.ascontiguousarray(
            np.repeat(is_tgt[None, r0:r1], NB, axis=0))
        in_maps.append(m)
    return in_maps


def _epilogue(raws, params, y_target):
    """raws: [B, R, 15] head outputs; mixture + loss in numpy."""
    g = lambda a: np.asarray(a, dtype=np.float32)
    h = raws[:, NKV:, :]                                  # [B, NT, 15]
    raw = h.reshape(B, NT, K, 1, 3)
    mean = raw[..., 0] + g(params["mean_bias"])[None, None, :, None]
    sp_in = raw[..., 1] + g(params["std_bias"])[None, None, :, None]
    std = np.minimum(np.logaddexp(0.0, sp_in), 2.0) + STD_MIN
    wl = raw[..., 2] + g(params["weight_bias"])[None, None, :, None]
    wmax = wl.max(axis=2, keepdims=True)
    we = np.exp(wl - wmax)
    w = we / we.sum(axis=2, keepdims=True)
    yt = g(y_target)[:, :, None, None]
    logp = (-0.5 * (math.log(2 * math.pi) + 2 * np.log(std)
                    + ((yt - mean) / std) ** 2)
            + np.log(np.clip(w, 1e-12, None)))
    lmax = logp.max(axis=2, keepdims=True)
    ll = np.log(np.exp(logp - lmax).sum(axis=2)) + lmax[:, :, 0, :]
    loss = -ll.mean()
    return (np.float32(loss), mean.astype(np.float32),
            std.astype(np.float32), w.astype(np.float32))


def kernel(params, x_context, y_context, x_buffer, y_buffer,
           x_target, y_target, mask_features, mask_rows):
    global _COMPILED
    if _COMPILED is None:
        _COMPILED = _build()
    in_maps = _prep_inputs(params, x_context, y_context, x_buffer, y_buffer,
                           x_target, y_target)
    res = run_bass_kernel_spmd(_COMPILED, in_maps, core_ids=list(range(N_CORES)))
    raws = np.zeros((B, R, 15), np.float32)
    for core in range(N_CORES):
        b, p = core // 4, core % 4
        raws[b, p * ROWS:(p + 1) * ROWS, :] = res.results[core]["raw"].T
    return _epilogue(raws, params, y_target)


# revision 15
# speedup vs baseline: 1.2603x; 1.2603x over previous
"""ARTabPFN forward kernel for 8 TRN2 NeuronCores.

Sharding: 2 batch groups x 4-way row sharding (386 rows/core).
Device does: 4 transformer layers + final norm + head MLP.
Host does: embedding (tiny) and the K=5 mixture/loss epilogue (tiny).

Activations live transposed in SBUF: [feature_on_partitions, rows_on_free].
All matmuls run as float32r (full PE speed, ~1e-4 rounding).
Row attention exploits sparsity: every row attends only to the first
NC+NB=1032 columns (ctx+buf), plus a masked self-term for target rows,
so K/V are gathered per layer only for those 1032 rows (AllGather over
each 4-core group).
"""

import math
import numpy as np
import ml_dtypes

import concourse.bass as bass
import concourse.bacc as bacc
import concourse.tile as tile
import concourse.mybir as mybir
from concourse.bass_utils import run_bass_kernel_spmd

F32R = mybir.dt.bfloat16  # compute dtype (bf16: FWL fast weight load, half DMA/AG bytes)
F32 = mybir.dt.float32
AF = mybir.ActivationFunctionType

B, NC, NB, NT, CF = 2, 1024, 8, 512, 32
D, H, L, DFF, K = 512, 8, 4, 1024, 5
R = NC + NB + NT          # 1544
NKV = NC + NB             # 1032
ROWS = R // 4             # 386 rows per core
DH = D // H               # 64
N_CORES = 8
STD_MIN = 1e-3

# peer p in a group owns rows [p*ROWS, (p+1)*ROWS); kv rows are < NKV
PEER_KV = [min(max(NKV - p * ROWS, 0), ROWS) for p in range(4)]   # [386,386,260,0]
CONTRIB_ROWS = D + ROWS   # 898: rows 0:512 = kT (cols 0:386), rows 512:898 = V row-major
KCHUNKS = [(s, min(128, NKV - s)) for s in range(0, NKV, 128)]    # 9 chunks, last = 8

_COMPILED = None


def _build():
    nc = bacc.Bacc("TRN2", target_bir_lowering=False, debug=False,
                   num_devices=N_CORES)

    def din(name, shape, dt=F32R):
        return nc.declare_dram_parameter(name, list(shape), dt, isOutput=False)

    x0 = din("x0", [D, ROWS])
    cones = din("cones", [128, 1])
    onesrow = din("onesrow", [1, 128])
    blockones = din("blockones", [128, 32])
    expander = din("expander", [8, D])
    sel8 = din("sel8", [1, 64])
    eye8 = din("eye8", [8, 8])
    vones = din("vones", [128, 8])
    bufmask = din("bufmask", [NB, ROWS])
    istgt8 = din("istgt8", [NB, ROWS])

    LW = []
    for l in range(L):
        LW.append({
            "wfvo": din(f"wfvo{l}", [D, D]), "bfvo": din(f"bfvo{l}", [D, 1], F32),
            "wq": din(f"wq{l}", [D, D]), "bq": din(f"bq{l}", [D, 1], F32),
            "wk": din(f"wk{l}", [D, D]),
            "wv": din(f"wv{l}", [D, D]),
            "wo": din(f"wo{l}", [D, D]), "bo": din(f"bo{l}", [D, 1], F32),
            "wf1": din(f"wf1_{l}", [D, DFF]), "bf1": din(f"bf1_{l}", [DFF, 1], F32),
            "wf2": din(f"wf2_{l}", [DFF, D]), "bf2": din(f"bf2_{l}", [D, 1], F32),
            "g1": din(f"g1_{l}", [D, 1], F32), "c1": din(f"c1_{l}", [D, 1], F32),
            "g2": din(f"g2_{l}", [D, 1], F32), "c2": din(f"c2_{l}", [D, 1], F32),
            "g3": din(f"g3_{l}", [D, 1], F32), "c3": din(f"c3_{l}", [D, 1], F32),
        })
    wh1 = din("wh1", [D, DFF]); bh1 = din("bh1", [DFF, 1], F32)
    wh2 = din("wh2", [DFF, 15]); bh2 = din("bh2", [15, 1], F32)
    gf = din("gf", [D, 1], F32); cf = din("cf", [D, 1], F32)

    raw_out = nc.declare_dram_parameter("raw", [15, ROWS], F32, isOutput=True)

    with tile.TileContext(nc) as tc:
        ctx_lp = nc.allow_low_precision(reason="deliberate f32r compute")
        ctx_lp.__enter__()
        with tc.tile_pool(name="const", bufs=1) as constp, \
             tc.tile_pool(name="acts", bufs=1) as actp, \
             tc.tile_pool(name="wd", bufs=2) as wdp, \
             tc.tile_pool(name="wff", bufs=1) as wffp, \
             tc.tile_pool(name="wvp", bufs=1) as wvp, \
             tc.tile_pool(name="kv", bufs=1) as kvp, \
             tc.tile_pool(name="attn", bufs=2) as atp, \
             tc.tile_pool(name="pvp", bufs=1) as pvp, \
             tc.tile_pool(name="small", bufs=1) as smp, \
             tc.tile_pool(name="pp", bufs=2, space="PSUM") as pp, \
             tc.tile_pool(name="ps", bufs=2, space="PSUM") as ps, \
             tc.tile_pool(name="pav", bufs=2, space="PSUM") as pav, \
             tc.tile_pool(name="pt", bufs=2, space="PSUM") as pt, \
             tc.tile_pool(name="dram", bufs=2, space="DRAM") as dramp:

            # ---- constants ----
            t_cones = constp.tile([128, 1], F32R, tag="cones")
            nc.sync.dma_start(t_cones[:], cones[:])
            t_onesrow = constp.tile([1, 128], F32R, tag="onesrow")
            nc.sync.dma_start(t_onesrow[:], onesrow[:])
            t_blko = constp.tile([128, 32], F32R, tag="blko")
            nc.sync.dma_start(t_blko[:], blockones[:])
            t_exp = constp.tile([8, D], F32R, tag="exp")
            nc.sync.dma_start(t_exp[:], expander[:])
            t_sel8 = constp.tile([1, 64], F32R, tag="sel8")
            nc.sync.dma_start(t_sel8[:], sel8[:])
            t_eye8 = constp.tile([8, 8], F32R, tag="eye8")
            nc.sync.dma_start(t_eye8[:], eye8[:])
            t_bufm = constp.tile([NB, ROWS], F32R, tag="bufm")
            nc.sync.dma_start(t_bufm[:], bufmask[:])
            t_istgt = constp.tile([NB, ROWS], F32R, tag="istgt")
            nc.sync.dma_start(t_istgt[:], istgt8[:])

            def load_w(drh, dout, pool, tag):
                """Load [D_in, dout] weights as tiles of [128, dout]."""
                nin = drh.shape[0]
                ts = []
                for ci in range(nin // 128):
                    t = pool.tile([128, dout], F32R, tag=f"{tag}{ci}")
                    nc.sync.dma_start(t[:], drh[ci * 128:(ci + 1) * 128, :])
                    ts.append(t)
                return ts

            def load_b(drh, tag, n=D):
                ts = []
                for ci in range(n // 128):
                    t = smp.tile([128, 1], F32, tag=f"{tag}{ci}")
                    nc.sync.dma_start(t[:], drh[ci * 128:(ci + 1) * 128, :])
                    ts.append(t)
                return ts

            def proj_t(xin, wts, epi, dout):
                """Transposed projection: out[co][128, ROWS] = sum_ci W[ci][:,co].T @ xin[ci].
                epi(psum, co) -> sbuf tile."""
                outs = []
                for co in range(dout // 128):
                    psum = ps.tile([128, ROWS], F32, tag="ps")
                    nci = len(xin)
                    for ci in range(nci):
                        nc.tensor.matmul(psum[:], wts[ci][:, co * 128:(co + 1) * 128],
                                         xin[ci][:], start=(ci == 0), stop=(ci == nci - 1))
                    outs.append(epi(psum, co))
                return outs

            def layernorm(xin, gts, cts, tag):
                """LN over features (partitions). xin: 4 tiles [128, ROWS] f32r."""
                pst = pt.tile([15, ROWS], F32, tag="pt")
                for ci in range(4):
                    nc.tensor.matmul(pst[0:1, :], t_cones[:], xin[ci][:],
                                     start=(ci == 0), stop=(ci == 3))
                mu = smp.tile([1, ROWS], F32, tag="mu")
                nc.scalar.activation(mu[:], pst[0:1, :], AF.Copy, scale=1.0 / D)
                psq = pt.tile([15, ROWS], F32, tag="pt")
                for ci in range(4):
                    sq = actp.tile([128, ROWS], F32R, tag="lnsq")
                    nc.vector.tensor_mul(sq[:], xin[ci][:], xin[ci][:])
                    nc.tensor.matmul(psq[0:1, :], t_cones[:], sq[:],
                                     start=(ci == 0), stop=(ci == 3))
                exq = smp.tile([1, ROWS], F32, tag="exq")
                nc.scalar.activation(exq[:], psq[0:1, :], AF.Copy, scale=1.0 / D)
                var = smp.tile([1, ROWS], F32, tag="var")
                nc.vector.tensor_mul(var[:], mu[:], mu[:])
                nc.vector.tensor_sub(var[:], exq[:], var[:])
                nc.vector.tensor_scalar_add(var[:], var[:], 1e-5)
                nc.scalar.activation(exq[:], var[:], AF.Sqrt)
                rr = smp.tile([1, ROWS], F32R, tag="rr")
                nc.vector.reciprocal(rr[:], exq[:])
                aa = smp.tile([1, ROWS], F32R, tag="aa")
                nc.vector.tensor_mul(aa[:], mu[:], rr[:])
                nc.vector.tensor_scalar_mul(aa[:], aa[:], -1.0)
                # broadcast r, a across partitions via K=1 matmul
                pbs = ps.tile([128, ROWS], F32, tag="ps")
                nc.tensor.matmul(pbs[:], t_onesrow[:], rr[:], start=True, stop=True)
                pba = ps.tile([128, ROWS], F32, tag="ps")
                nc.tensor.matmul(pba[:], t_onesrow[:], aa[:], start=True, stop=True)
                outs = []
                for ci in range(4):
                    t = actp.tile([128, ROWS], F32, tag="lnt")
                    nc.vector.tensor_mul(t[:], xin[ci][:], pbs[:])
                    nc.vector.tensor_add(t[:], t[:], pba[:])
                    o = actp.tile([128, ROWS], F32R, tag=f"{tag}{ci}")
                    nc.scalar.activation(o[:], t[:], AF.Identity,
                                         bias=cts[ci][:], scale=gts[ci][:])
                    outs.append(o)
                return outs

            # ---- initial activations ----
            x = []
            for ci in range(4):
                t = actp.tile([128, ROWS], F32R, tag=f"x{ci}")
                nc.sync.dma_start(t[:], x0[ci * 128:(ci + 1) * 128, :])
                x.append(t)

            for l in range(L):
                lw = LW[l]
                w_fvo = load_w(lw["wfvo"], D, wdp, "wd")
                b_fvo = load_b(lw["bfvo"], "bfvo")
                g1 = load_b(lw["g1"], "g1"); c1 = load_b(lw["c1"], "c1")

                # ---- feature attention (collapsed) + residual + LN1 ----
                def epi_fattn(psum, co):
                    t = actp.tile([128, ROWS], F32R, tag=f"res{co}")
                    nc.scalar.activation(t[:], psum[:], AF.Identity, bias=b_fvo[co][:])
                    nc.vector.tensor_add(t[:], t[:], x[co][:])
                    return t
                r1 = proj_t(x, w_fvo, epi_fattn, D)
                xf = layernorm(r1, g1, c1, "xf")

                # ---- K (no bias: cancels in softmax) and V row-major (bias folded into o-proj) ----
                w_k = load_w(lw["wk"], D, wdp, "wd")

                def epi_plain(tag):
                    def f(psum, co):
                        t = actp.tile([128, ROWS], F32R, tag=f"{tag}{co}")
                        nc.vector.tensor_copy(t[:], psum[:])
                        return t
                    return f
                kT = proj_t(xf, w_k, epi_plain("kT"), D)

                w_v = load_w(lw["wv"], D, wvp, "wv")
                contrib = dramp.tile([CONTRIB_ROWS, D], F32R, tag="contrib")
                gathered = dramp.tile([4 * CONTRIB_ROWS, D], F32R, tag="gathered")
                for ci in range(4):
                    nc.sync.dma_start(contrib[ci * 128:(ci + 1) * 128, 0:ROWS], kT[ci][:])
                # V row-major: out[row_chunk, 512]
                vrm = []
                for rt in range(4):
                    rc = min(128, ROWS - rt * 128)
                    psum = pp.tile([128, D], F32, tag="pp")
                    for ci in range(4):
                        nc.tensor.matmul(psum[0:rc, :],
                                         xf[ci][:, rt * 128:rt * 128 + rc],
                                         w_v[ci][:], start=(ci == 0), stop=(ci == 3))
                    t = actp.tile([128, D], F32R, tag=f"vrm{rt}")
                    nc.vector.tensor_copy(t[0:rc, :], psum[0:rc, :])
                    vrm.append((t, rc))
                    nc.sync.dma_start(
                        contrib[D + rt * 128:D + rt * 128 + rc, :], t[0:rc, :])

                nc.gpsimd.collective_compute(
                    "AllGather", mybir.AluOpType.bypass,
                    ins=[contrib.opt()], outs=[gathered.opt()],
                    replica_groups=[[0, 1, 2, 3], [4, 5, 6, 7]])

                # ---- Q (with bias), vT (transposed V for self term) while gather flies ----
                w_q = load_w(lw["wq"], D, wdp, "wd")
                b_q = load_b(lw["bq"], "bq")

                def epi_q(psum, co):
                    t = actp.tile([128, ROWS], F32R, tag=f"qT{co}")
                    nc.scalar.activation(t[:], psum[:], AF.Identity, bias=b_q[co][:])
                    return t
                qT = proj_t(xf, w_q, epi_q, D)
                vT = proj_t(xf, w_v, epi_plain("vT"), D)

                # self scores: s_self[h, row] = sum_dh qT*kT
                pss = pt.tile([15, ROWS], F32, tag="pt")
                for ci in range(4):
                    t = actp.tile([128, ROWS], F32R, tag="qk")
                    nc.vector.tensor_mul(t[:], qT[ci][:], kT[ci][:])
                    nc.tensor.matmul(pss[0:8, :], t_blko[:, ci * 8:(ci + 1) * 8],
                                     t[:], start=(ci == 0), stop=(ci == 3))
                sst = smp.tile([15, ROWS], F32, tag="rawt")
                nc.vector.tensor_copy(sst[0:8, :], pss[0:8, :])
                pself = smp.tile([8, ROWS], F32R, tag="pself")
                nc.scalar.activation(pself[:], sst[0:8, :], AF.Exp, scale=1.0 / math.sqrt(DH))
                nc.vector.tensor_mul(pself[:], pself[:], t_istgt[:])

                # ---- repack gathered K/V ----
                kg = []
                for ci in range(4):
                    t = kvp.tile([128, NKV], F32R, tag=f"kg{ci}")
                    off = 0
                    for p in range(4):
                        n = PEER_KV[p]
                        if n == 0:
                            continue
                        nc.sync.dma_start(
                            t[:, off:off + n],
                            gathered[p * CONTRIB_ROWS + ci * 128:
                                     p * CONTRIB_ROWS + (ci + 1) * 128, 0:n])
                        off += n
                    kg.append(t)
                vg = []
                for (s, n) in KCHUNKS:
                    t = kvp.tile([128, 8, DH + 1], F32R, tag=f"vg{s}")
                    nc.sync.dma_start(t[0:n, :, DH:DH + 1], vones[0:n, :])
                    # rows s..s+n map onto peer blocks
                    r0 = s
                    doff = 0
                    while r0 < s + n:
                        p = min(r0 // ROWS, 2)
                        lr0 = r0 - p * ROWS
                        cnt = min(PEER_KV[p] - lr0, s + n - r0)
                        src = gathered[p * CONTRIB_ROWS + D + lr0:
                                       p * CONTRIB_ROWS + D + lr0 + cnt, :]
                        nc.sync.dma_start(
                            t[doff:doff + cnt, :, 0:DH],
                            src.rearrange("p (h d) -> p h d", h=8))
                        doff += cnt
                        r0 += cnt
                    vg.append((t, n))

                # ---- attention per head ----
                pvs = []
                dns = []
                for h in range(H):
                    ci, off = h // 2, (h % 2) * 64
                    pchunks = []
                    for ki, (s, n) in enumerate(KCHUNKS):
                        psc = ps.tile([128, ROWS], F32, tag="ps")
                        nc.tensor.matmul(psc[0:n, :],
                                         kg[ci][off:off + 64, s:s + n],
                                         qT[ci][off:off + 64, :],
                                         start=True, stop=True)
                        pe = atp.tile([128, ROWS], F32R, tag="pch")
                        nc.scalar.activation(pe[0:n, :], psc[0:n, :], AF.Exp,
                                             scale=1.0 / math.sqrt(DH))
                        if s == NC:  # buffer columns: causal/visibility mask
                            nc.vector.tensor_mul(pe[0:n, :], pe[0:n, :], t_bufm[:])
                        pchunks.append((pe, n))
                    pavt = pav.tile([DH + 1, ROWS], F32, tag="pav")
                    for ki, ((vt, n), (pe, n2)) in enumerate(zip(vg, pchunks)):
                        nc.tensor.matmul(pavt[:], vt[0:n, h, :], pe[0:n, :],
                                         start=(ki == 0), stop=(ki == len(vg) - 1))
                    pv = pvp.tile([128, ROWS], F32, tag=f"pv{h}")
                    nc.vector.tensor_copy(pv[(h % 2) * 64:(h % 2) * 64 + 64, :],
                                          pavt[0:DH, :])
                    dn = pvp.tile([1, ROWS], F32R, tag=f"dn{h}")
                    nc.vector.tensor_copy(dn[:], pavt[DH:DH + 1, :])
                    pvs.append(pv)
                    dns.append(dn)

                # denominators: assemble [8, ROWS] in psum, add self term
                pden = pt.tile([15, ROWS], F32, tag="pt")
                for h in range(H):
                    nc.tensor.matmul(pden[0:8, :], t_sel8[:, h * 8:(h + 1) * 8],
                                     dns[h][:], start=(h == 0), stop=False)
                nc.tensor.matmul(pden[0:8, :], t_eye8[:], pself[:],
                                 start=False, stop=True)
                rec = smp.tile([8, ROWS], F32R, tag="rec")
                nc.vector.reciprocal(rec[:], pden[0:8, :])
                selfw = smp.tile([8, ROWS], F32R, tag="selfw")
                nc.vector.tensor_mul(selfw[:], pself[:], rec[:])

                attn = []
                for ci in range(4):
                    prb = ps.tile([128, ROWS], F32, tag="ps")
                    nc.tensor.matmul(prb[:], t_exp[:, ci * 128:(ci + 1) * 128],
                                     rec[:], start=True, stop=True)
                    psw = ps.tile([128, ROWS], F32, tag="ps")
                    nc.tensor.matmul(psw[:], t_exp[:, ci * 128:(ci + 1) * 128],
                                     selfw[:], start=True, stop=True)
                    t = actp.tile([128, ROWS], F32R, tag=f"attn{ci}")
                    nc.vector.tensor_mul(t[:], vT[ci][:], psw[:])
                    t2 = actp.tile([128, ROWS], F32, tag="t2")
                    for hh in range(2):
                        o = hh * 64
                        nc.vector.tensor_mul(t2[o:o + 64, :],
                                             pvs[2 * ci + hh][o:o + 64, :],
                                             prb[o:o + 64, :])
                        nc.vector.tensor_add(t[o:o + 64, :],
                                             t[o:o + 64, :], t2[o:o + 64, :])
                    attn.append(t)

                # ---- o-proj + residual + LN2 ----
                w_o = load_w(lw["wo"], D, wdp, "wd")
                b_o = load_b(lw["bo"], "bo")
                g2 = load_b(lw["g2"], "g2"); c2 = load_b(lw["c2"], "c2")

                def epi_o(psum, co):
                    t = actp.tile([128, ROWS], F32R, tag=f"res{co}")
                    nc.scalar.activation(t[:], psum[:], AF.Identity, bias=b_o[co][:])
                    nc.vector.tensor_add(t[:], t[:], xf[co][:])
                    return t
                r2 = proj_t(attn, w_o, epi_o, D)
                x2 = layernorm(r2, g2, c2, "x2")

                # ---- MLP + residual + LN3 ----
                w_f1 = load_w(lw["wf1"], DFF, wffp, "wf")
                b_f1 = load_b(lw["bf1"], "bf1", DFF)

                def epi_g(psum, co):
                    t = actp.tile([128, ROWS], F32R, tag=f"h1_{co}")
                    nc.scalar.activation(t[:], psum[:], AF.Gelu, bias=b_f1[co][:])
                    return t
                h1 = proj_t(x2, w_f1, epi_g, DFF)

                w_f2 = load_w(lw["wf2"], D, wffp, "wf")
                b_f2 = load_b(lw["bf2"], "bf2")
                g3 = load_b(lw["g3"], "g3"); c3 = load_b(lw["c3"], "c3")

                def epi_f2(psum, co):
                    t = actp.tile([128, ROWS], F32R, tag=f"res{co}")
                    nc.scalar.activation(t[:], psum[:], AF.Identity, bias=b_f2[co][:])
                    nc.vector.tensor_add(t[:], t[:], x2[co][:])
                    return t
                r3 = proj_t(h1, w_f2, epi_f2, D)
                x = layernorm(r3, g3, c3, "x")

            # ---- final norm + head ----
            gft = load_b(gf, "gf"); cft = load_b(cf, "cf")
            z = layernorm(x, gft, cft, "res")
            w_h1 = load_w(wh1, DFF, wffp, "wf")
            b_h1 = load_b(bh1, "bh1", DFF)

            def epi_h1(psum, co):
                t = actp.tile([128, ROWS], F32R, tag=f"h1_{co}")
                nc.scalar.activation(t[:], psum[:], AF.Gelu, bias=b_h1[co][:])
                return t
            hh = proj_t(z, w_h1, epi_h1, DFF)

            w_h2 = load_w(wh2, 15, wffp, "wf")
            b_h2t = smp.tile([15, 1], F32, tag="bh2")
            nc.sync.dma_start(b_h2t[:], bh2[:])
            ph2 = pt.tile([15, ROWS], F32, tag="pt")
            for ci in range(8):
                nc.tensor.matmul(ph2[:], w_h2[ci][:], hh[ci][:],
                                 start=(ci == 0), stop=(ci == 7))
            rawt = smp.tile([15, ROWS], F32, tag="rawt")
            nc.scalar.activation(rawt[:], ph2[:], AF.Identity, bias=b_h2t[:])
            nc.sync.dma_start(raw_out[:], rawt[:])

    nc.compile()
    return nc


def _prep_inputs(params, x_context, y_context, x_buffer, y_buffer,
                 x_target, y_target):
    g = lambda a: np.asarray(a, dtype=np.float32)

    # ---- host embedding (tiny: ~1.6M FLOPs) ----
    wx, bx = g(params["x_embed"]["W"]), g(params["x_embed"]["b"])
    wy, by = g(params["y_embed"]["W"]), g(params["y_embed"]["b"])
    marker = g(params["marker"])
    ar = g(params["ar_tokens"])
    def emb(xx, yy=None):
        e = g(xx).mean(axis=2, keepdims=True) @ wx + bx
        if yy is not None:
            e = e + g(yy)[..., None] @ wy + by
        return e
    ctx = emb(x_context, y_context) + marker[1]
    buf = emb(x_buffer, y_buffer) + marker[2] + ar
    tgt = emb(x_target) + marker[0]
    x = np.concatenate([ctx, buf, tgt], axis=1)          # [B, R, D]

    # ---- per-layer fused weights ----
    layers = []
    for lp in params["layers"]:
        wvf, bvf = g(lp["attn_f"]["v"]["W"]), g(lp["attn_f"]["v"]["b"])
        wof, bof = g(lp["attn_f"]["o"]["W"]), g(lp["attn_f"]["o"]["b"])
        wo, bo = g(lp["attn_r"]["o"]["W"]), g(lp["attn_r"]["o"]["b"])
        bv = g(lp["attn_r"]["v"]["b"])
        layers.append({
            "wfvo": wvf @ wof, "bfvo": (bvf @ wof + bof)[:, None],
            "wq": g(lp["attn_r"]["q"]["W"]), "bq": g(lp["attn_r"]["q"]["b"])[:, None],
            "wk": g(lp["attn_r"]["k"]["W"]),
            "wv": g(lp["attn_r"]["v"]["W"]),
            "wo": wo, "bo": (bv @ wo + bo)[:, None],
            "wf1": g(lp["ff1"]["W"]), "bf1": g(lp["ff1"]["b"])[:, None],
            "wf2": g(lp["ff2"]["W"]), "bf2": g(lp["ff2"]["b"])[:, None],
            "g1": g(lp["n1"]["g"])[:, None], "c1": g(lp["n1"]["b"])[:, None],
            "g2": g(lp["n2"]["g"])[:, None], "c2": g(lp["n2"]["b"])[:, None],
            "g3": g(lp["n3"]["g"])[:, None], "c3": g(lp["n3"]["b"])[:, None],
        })

    # ---- masks / constants ----
    idx = np.arange(R)
    is_tgt = (idx >= NKV).astype(np.float32)
    # buffer-column visibility (cols NC..NC+NB) per query row
    bm = np.zeros((R, NB), np.float32)
    bcol = NC + np.arange(NB)
    is_buf_row = (idx >= NC) & (idx < NKV)
    bm[is_buf_row[:, None] & (bcol[None, :] <= idx[:, None])] = 1.0
    bm[idx >= NKV, :] = 1.0

    expander = np.zeros((8, D), np.float32)
    for m in range(D):
        expander[m // DH, m] = 1.0
    blockones = np.zeros((128, 32), np.float32)
    for ci in range(4):
        blockones[0:64, ci * 8 + 2 * ci] = 1.0
        blockones[64:128, ci * 8 + 2 * ci + 1] = 1.0

    sel8 = np.zeros((1, 64), np.float32)
    for h in range(8):
        sel8[0, h * 8 + h] = 1.0
    common = {
        "sel8": sel8,
        "eye8": np.eye(8, dtype=np.float32),
        "vones": np.ones((128, 8), np.float32),
        "cones": np.ones((128, 1), np.float32),
        "onesrow": np.ones((1, 128), np.float32),
        "blockones": blockones,
        "expander": expander,
        "wh1": g(params["head1"]["W"]), "bh1": g(params["head1"]["b"])[:, None],
        "wh2": g(params["head2"]["W"]), "bh2": g(params["head2"]["b"])[:, None],
        "gf": g(params["final_norm"]["g"])[:, None],
        "cf": g(params["final_norm"]["b"])[:, None],
    }
    for l, lw in enumerate(layers):
        for k, v in lw.items():
            name = f"{k}{l}" if not k[-1].isdigit() else f"{k}_{l}"
            common[name] = np.ascontiguousarray(v)

    # bf16 for everything except per-partition bias/scale vectors (f32 in graph)
    f32_names = {"bfvo", "bq", "bo", "bf1", "bf2", "g1", "c1", "g2", "c2",
                 "g3", "c3", "bh", "bh1", "bh2", "gf", "cf"}
    def is_f32(name):
        base = name.rstrip("0123456789").rstrip("_")
        return base in f32_names
    common = {k: (v if is_f32(k) else v.astype(ml_dtypes.bfloat16))
              for k, v in common.items()}
    in_maps = []
    for core in range(N_CORES):
        b, p = core // 4, core % 4
        r0, r1 = p * ROWS, (p + 1) * ROWS
        m = dict(common)
        m["x0"] = np.ascontiguousarray(x[b, r0:r1, :].T).astype(ml_dtypes.bfloat16)
        m["bufmask"] = np.ascontiguousarray(bm[r0:r1, :].T).astype(ml_dtypes.bfloat16)
        m["istgt8"] = np.ascontiguousarray(
            np.repeat(is_tgt[None, r0:r1], NB, axis=0)).astype(ml_dtypes.bfloat16)
        in_maps.append(m)
    return in_maps


def _epilogue(raws, params, y_target):
    """raws: [B, R, 15] head outputs; mixture + loss in numpy."""
    g = lambda a: np.asarray(a, dtype=np.float32)
    h = raws[:, NKV:, :]                                  # [B, NT, 15]
    raw = h.reshape(B, NT, K, 1, 3)
    mean = raw[..., 0] + g(params["mean_bias"])[None, None, :, None]
    sp_in = raw[..., 1] + g(params["std_bias"])[None, None, :, None]
    std = np.minimum(np.logaddexp(0.0, sp_in), 2.0) + STD_MIN
    wl = raw[..., 2] + g(params["weight_bias"])[None, None, :, None]
    wmax = wl.max(axis=2, keepdims=True)
    we = np.exp(wl - wmax)
    w = we / we.sum(axis=2, keepdims=True)
    yt = g(y_target)[:, :, None, None]
    logp = (-0.5 * (math.log(2 * math.pi) + 2 * np.log(std)
                    + ((yt - mean) / std) ** 2)
            + np.log(np.clip(w, 1e-12, None)))
    lmax = logp.max(axis=2, keepdims=True)
    ll = np.log(np.exp(logp - lmax).sum(axis=2)) + lmax[:, :, 0, :]
    loss = -ll.mean()
    return (np.float32(loss), mean.astype(np.float32),
            std.astype(np.float32), w.astype(np.float32))


def kernel(params, x_context, y_context, x_buffer, y_buffer,
           x_target, y_target, mask_features, mask_rows):
    global _COMPILED
    if _COMPILED is None:
        _COMPILED = _build()
    in_maps = _prep_inputs(params, x_context, y_context, x_buffer, y_buffer,
                           x_target, y_target)
    res = run_bass_kernel_spmd(_COMPILED, in_maps, core_ids=list(range(N_CORES)))
    raws = np.zeros((B, R, 15), np.float32)
    for core in range(N_CORES):
        b, p = core // 4, core % 4
        raws[b, p * ROWS:(p + 1) * ROWS, :] = res.results[core]["raw"].T
    return _epilogue(raws, params, y_target)


# revision 17
# speedup vs baseline: 1.4859x; 1.1790x over previous
"""ARTabPFN forward kernel for 8 TRN2 NeuronCores.

Sharding: 2 batch groups x 4-way row sharding (386 rows/core).
Device does: 4 transformer layers + final norm + head MLP.
Host does: embedding (tiny) and the K=5 mixture/loss epilogue (tiny).

Activations live transposed in SBUF: [feature_on_partitions, rows_on_free].
All matmuls run as float32r (full PE speed, ~1e-4 rounding).
Row attention exploits sparsity: every row attends only to the first
NC+NB=1032 columns (ctx+buf), plus a masked self-term for target rows,
so K/V are gathered per layer only for those 1032 rows (AllGather over
each 4-core group).
"""

import math
import numpy as np
import ml_dtypes

import concourse.bass as bass
import concourse.bacc as bacc
import concourse.tile as tile
import concourse.mybir as mybir
from concourse.bass_utils import run_bass_kernel_spmd

F32R = mybir.dt.bfloat16  # compute dtype (bf16: FWL fast weight load, half DMA/AG bytes)
F32 = mybir.dt.float32
AF = mybir.ActivationFunctionType

B, NC, NB, NT, CF = 2, 1024, 8, 512, 32
D, H, L, DFF, K = 512, 8, 4, 1024, 5
R = NC + NB + NT          # 1544
NKV = NC + NB             # 1032
ROWS = R // 4             # 386 rows per core
DH = D // H               # 64
N_CORES = 8
STD_MIN = 1e-3

# peer p in a group owns rows [p*ROWS, (p+1)*ROWS); kv rows are < NKV
PEER_KV = [min(max(NKV - p * ROWS, 0), ROWS) for p in range(4)]   # [386,386,260,0]
CONTRIB_ROWS = D + ROWS   # legacy (unused)
KCHUNKS = [(s, min(128, NKV - s)) for s in range(0, NKV, 128)]    # 9 chunks, last = 8

_COMPILED = None


def _build():
    nc = bacc.Bacc("TRN2", target_bir_lowering=False, debug=False,
                   num_devices=N_CORES)

    def din(name, shape, dt=F32R):
        return nc.declare_dram_parameter(name, list(shape), dt, isOutput=False)

    x0 = din("x0", [D, ROWS])
    cones = din("cones", [128, 1])
    onesrow = din("onesrow", [1, 128])
    blockones = din("blockones", [128, 32])
    expander = din("expander", [8, D])
    sel8 = din("sel8", [1, 64])
    eye8 = din("eye8", [8, 8])
    vones = din("vones", [128, 8])
    bufmask = din("bufmask", [NB, ROWS])
    istgt8 = din("istgt8", [NB, ROWS])

    LW = []
    for l in range(L):
        LW.append({
            "wfvo": din(f"wfvo{l}", [D, D]), "bfvo": din(f"bfvo{l}", [D, 1], F32),
            "wq": din(f"wq{l}", [D, D]), "bq": din(f"bq{l}", [D, 1], F32),
            "wk": din(f"wk{l}", [D, D]),
            "wv": din(f"wv{l}", [D, D]),
            "wo": din(f"wo{l}", [D, D]), "bo": din(f"bo{l}", [D, 1], F32),
            "wf1": din(f"wf1_{l}", [D, DFF]), "bf1": din(f"bf1_{l}", [DFF, 1], F32),
            "wf2": din(f"wf2_{l}", [DFF, D]), "bf2": din(f"bf2_{l}", [D, 1], F32),
            "g1": din(f"g1_{l}", [D, 1], F32), "c1": din(f"c1_{l}", [D, 1], F32),
            "g2": din(f"g2_{l}", [D, 1], F32), "c2": din(f"c2_{l}", [D, 1], F32),
            "g3": din(f"g3_{l}", [D, 1], F32), "c3": din(f"c3_{l}", [D, 1], F32),
        })
    wh1 = din("wh1", [D, DFF]); bh1 = din("bh1", [DFF, 1], F32)
    wh2 = din("wh2", [DFF, 15]); bh2 = din("bh2", [15, 1], F32)
    gf = din("gf", [D, 1], F32); cf = din("cf", [D, 1], F32)

    raw_out = nc.declare_dram_parameter("raw", [15, ROWS], F32, isOutput=True)

    with tile.TileContext(nc) as tc:
        ctx_lp = nc.allow_low_precision(reason="deliberate f32r compute")
        ctx_lp.__enter__()
        with tc.tile_pool(name="const", bufs=1) as constp, \
             tc.tile_pool(name="acts", bufs=1) as actp, \
             tc.tile_pool(name="wd", bufs=2) as wdp, \
             tc.tile_pool(name="wff", bufs=1) as wffp, \
             tc.tile_pool(name="wvp", bufs=1) as wvp, \
             tc.tile_pool(name="kv", bufs=1) as kvp, \
             tc.tile_pool(name="attn", bufs=2) as atp, \
             tc.tile_pool(name="pvp", bufs=1) as pvp, \
             tc.tile_pool(name="small", bufs=1) as smp, \
             tc.tile_pool(name="pp", bufs=2, space="PSUM") as pp, \
             tc.tile_pool(name="ps", bufs=2, space="PSUM") as ps, \
             tc.tile_pool(name="pav", bufs=2, space="PSUM") as pav, \
             tc.tile_pool(name="pt", bufs=2, space="PSUM") as pt, \
             tc.tile_pool(name="dram", bufs=2, space="DRAM") as dramp:

            # ---- constants ----
            t_cones = constp.tile([128, 1], F32R, tag="cones")
            nc.sync.dma_start(t_cones[:], cones[:])
            t_onesrow = constp.tile([1, 128], F32R, tag="onesrow")
            nc.sync.dma_start(t_onesrow[:], onesrow[:])
            t_blko = constp.tile([128, 32], F32R, tag="blko")
            nc.sync.dma_start(t_blko[:], blockones[:])
            t_exp = constp.tile([8, D], F32R, tag="exp")
            nc.sync.dma_start(t_exp[:], expander[:])
            t_sel8 = constp.tile([1, 64], F32R, tag="sel8")
            nc.sync.dma_start(t_sel8[:], sel8[:])
            t_eye8 = constp.tile([8, 8], F32R, tag="eye8")
            nc.sync.dma_start(t_eye8[:], eye8[:])
            t_bufm = constp.tile([NB, ROWS], F32R, tag="bufm")
            nc.sync.dma_start(t_bufm[:], bufmask[:])
            t_istgt = constp.tile([NB, ROWS], F32R, tag="istgt")
            nc.sync.dma_start(t_istgt[:], istgt8[:])

            def load_w(drh, dout, pool, tag):
                """Load [D_in, dout] weights as tiles of [128, dout]."""
                nin = drh.shape[0]
                ts = []
                for ci in range(nin // 128):
                    t = pool.tile([128, dout], F32R, tag=f"{tag}{ci}")
                    nc.sync.dma_start(t[:], drh[ci * 128:(ci + 1) * 128, :])
                    ts.append(t)
                return ts

            def load_b(drh, tag, n=D):
                ts = []
                for ci in range(n // 128):
                    t = smp.tile([128, 1], F32, tag=f"{tag}{ci}")
                    nc.sync.dma_start(t[:], drh[ci * 128:(ci + 1) * 128, :])
                    ts.append(t)
                return ts

            def proj_t(xin, wts, epi, dout):
                """Transposed projection: out[co][128, ROWS] = sum_ci W[ci][:,co].T @ xin[ci].
                epi(psum, co) -> sbuf tile."""
                outs = []
                for co in range(dout // 128):
                    psum = ps.tile([128, ROWS], F32, tag="ps")
                    nci = len(xin)
                    for ci in range(nci):
                        nc.tensor.matmul(psum[:], wts[ci][:, co * 128:(co + 1) * 128],
                                         xin[ci][:], start=(ci == 0), stop=(ci == nci - 1))
                    outs.append(epi(psum, co))
                return outs

            def layernorm(xin, gts, cts, tag):
                """LN over features (partitions). xin: 4 tiles [128, ROWS] f32r."""
                pst = pt.tile([15, ROWS], F32, tag="pt")
                for ci in range(4):
                    nc.tensor.matmul(pst[0:1, :], t_cones[:], xin[ci][:],
                                     start=(ci == 0), stop=(ci == 3))
                mu = smp.tile([1, ROWS], F32, tag="mu")
                nc.scalar.activation(mu[:], pst[0:1, :], AF.Copy, scale=1.0 / D)
                psq = pt.tile([15, ROWS], F32, tag="pt")
                for ci in range(4):
                    sq = actp.tile([128, ROWS], F32R, tag="lnsq")
                    nc.vector.tensor_mul(sq[:], xin[ci][:], xin[ci][:])
                    nc.tensor.matmul(psq[0:1, :], t_cones[:], sq[:],
                                     start=(ci == 0), stop=(ci == 3))
                exq = smp.tile([1, ROWS], F32, tag="exq")
                nc.scalar.activation(exq[:], psq[0:1, :], AF.Copy, scale=1.0 / D)
                var = smp.tile([1, ROWS], F32, tag="var")
                nc.vector.tensor_mul(var[:], mu[:], mu[:])
                nc.vector.tensor_sub(var[:], exq[:], var[:])
                nc.vector.tensor_scalar_add(var[:], var[:], 1e-5)
                nc.scalar.activation(exq[:], var[:], AF.Sqrt)
                rr = smp.tile([1, ROWS], F32R, tag="rr")
                nc.vector.reciprocal(rr[:], exq[:])
                aa = smp.tile([1, ROWS], F32R, tag="aa")
                nc.vector.tensor_mul(aa[:], mu[:], rr[:])
                nc.vector.tensor_scalar_mul(aa[:], aa[:], -1.0)
                # broadcast r, a across partitions via K=1 matmul
                pbs = ps.tile([128, ROWS], F32, tag="ps")
                nc.tensor.matmul(pbs[:], t_onesrow[:], rr[:], start=True, stop=True)
                pba = ps.tile([128, ROWS], F32, tag="ps")
                nc.tensor.matmul(pba[:], t_onesrow[:], aa[:], start=True, stop=True)
                outs = []
                for ci in range(4):
                    t = actp.tile([128, ROWS], F32, tag="lnt")
                    nc.vector.tensor_mul(t[:], xin[ci][:], pbs[:])
                    nc.vector.tensor_add(t[:], t[:], pba[:])
                    o = actp.tile([128, ROWS], F32R, tag=f"{tag}{ci}")
                    nc.scalar.activation(o[:], t[:], AF.Identity,
                                         bias=cts[ci][:], scale=gts[ci][:])
                    outs.append(o)
                return outs

            # ---- initial activations ----
            x = []
            for ci in range(4):
                t = actp.tile([128, ROWS], F32R, tag=f"x{ci}")
                nc.sync.dma_start(t[:], x0[ci * 128:(ci + 1) * 128, :])
                x.append(t)

            for l in range(L):
                lw = LW[l]
                w_fvo = load_w(lw["wfvo"], D, wdp, "wd")
                b_fvo = load_b(lw["bfvo"], "bfvo")
                g1 = load_b(lw["g1"], "g1"); c1 = load_b(lw["c1"], "c1")

                # ---- feature attention (collapsed) + residual + LN1 ----
                def epi_fattn(psum, co):
                    t = actp.tile([128, ROWS], F32R, tag=f"res{co}")
                    nc.scalar.activation(t[:], psum[:], AF.Identity, bias=b_fvo[co][:])
                    nc.vector.tensor_add(t[:], t[:], x[co][:])
                    return t
                r1 = proj_t(x, w_fvo, epi_fattn, D)
                xf = layernorm(r1, g1, c1, "xf")

                # ---- K (no bias: cancels in softmax) and V row-major (bias folded into o-proj) ----
                w_k = load_w(lw["wk"], D, wdp, "wd")

                def epi_plain(tag):
                    def f(psum, co):
                        t = actp.tile([128, ROWS], F32R, tag=f"{tag}{co}")
                        nc.vector.tensor_copy(t[:], psum[:])
                        return t
                    return f
                kT = proj_t(xf, w_k, epi_plain("kT"), D)

                w_v = load_w(lw["wv"], D, wvp, "wv")
                contribk = dramp.tile([D, ROWS], F32R, tag="contribk")
                gatheredk = dramp.tile([4 * D, ROWS], F32R, tag="gatheredk")
                contribv = dramp.tile([ROWS, D], F32R, tag="contribv")
                gatheredv = dramp.tile([4 * ROWS, D], F32R, tag="gatheredv")
                for ci in range(4):
                    nc.sync.dma_start(contribk[ci * 128:(ci + 1) * 128, :], kT[ci][:])
                nc.gpsimd.collective_compute(
                    "AllGather", mybir.AluOpType.bypass,
                    ins=[contribk.opt()], outs=[gatheredk.opt()],
                    replica_groups=[[0, 1, 2, 3], [4, 5, 6, 7]])
                # V row-major: out[row_chunk, 512]
                vrm = []
                for rt in range(4):
                    rc = min(128, ROWS - rt * 128)
                    psum = pp.tile([128, D], F32, tag="pp")
                    for ci in range(4):
                        nc.tensor.matmul(psum[0:rc, :],
                                         xf[ci][:, rt * 128:rt * 128 + rc],
                                         w_v[ci][:], start=(ci == 0), stop=(ci == 3))
                    t = actp.tile([128, D], F32R, tag=f"vrm{rt}")
                    nc.vector.tensor_copy(t[0:rc, :], psum[0:rc, :])
                    vrm.append((t, rc))
                    nc.sync.dma_start(contribv[rt * 128:rt * 128 + rc, :], t[0:rc, :])
                nc.gpsimd.collective_compute(
                    "AllGather", mybir.AluOpType.bypass,
                    ins=[contribv.opt()], outs=[gatheredv.opt()],
                    replica_groups=[[0, 1, 2, 3], [4, 5, 6, 7]])

                # ---- Q (with bias), vT (transposed V for self term) while gather flies ----
                w_q = load_w(lw["wq"], D, wdp, "wd")
                b_q = load_b(lw["bq"], "bq")

                def epi_q(psum, co):
                    t = actp.tile([128, ROWS], F32R, tag=f"qT{co}")
                    nc.scalar.activation(t[:], psum[:], AF.Identity, bias=b_q[co][:])
                    return t
                qT = proj_t(xf, w_q, epi_q, D)
                vT = proj_t(xf, w_v, epi_plain("vT"), D)

                # self scores: s_self[h, row] = sum_dh qT*kT
                pss = pt.tile([15, ROWS], F32, tag="pt")
                for ci in range(4):
                    t = actp.tile([128, ROWS], F32R, tag="qk")
                    nc.vector.tensor_mul(t[:], qT[ci][:], kT[ci][:])
                    nc.tensor.matmul(pss[0:8, :], t_blko[:, ci * 8:(ci + 1) * 8],
                                     t[:], start=(ci == 0), stop=(ci == 3))
                sst = smp.tile([15, ROWS], F32, tag="rawt")
                nc.vector.tensor_copy(sst[0:8, :], pss[0:8, :])
                pself = smp.tile([8, ROWS], F32R, tag="pself")
                nc.scalar.activation(pself[:], sst[0:8, :], AF.Exp, scale=1.0 / math.sqrt(DH))
                nc.vector.tensor_mul(pself[:], pself[:], t_istgt[:])

                # ---- repack gathered K/V ----
                kg = []
                for ci in range(4):
                    t = kvp.tile([128, NKV], F32R, tag=f"kg{ci}")
                    off = 0
                    for p in range(4):
                        n = PEER_KV[p]
                        if n == 0:
                            continue
                        nc.sync.dma_start(
                            t[:, off:off + n],
                            gatheredk[p * D + ci * 128:p * D + (ci + 1) * 128, 0:n])
                        off += n
                    kg.append(t)
                vg = []
                for (s, n) in KCHUNKS:
                    t = kvp.tile([128, 8, DH + 1], F32R, tag=f"vg{s}")
                    nc.sync.dma_start(t[0:n, :, DH:DH + 1], vones[0:n, :])
                    # rows s..s+n map onto peer blocks
                    r0 = s
                    doff = 0
                    while r0 < s + n:
                        p = min(r0 // ROWS, 2)
                        lr0 = r0 - p * ROWS
                        cnt = min(PEER_KV[p] - lr0, s + n - r0)
                        src = gatheredv[p * ROWS + lr0:p * ROWS + lr0 + cnt, :]
                        nc.sync.dma_start(
                            t[doff:doff + cnt, :, 0:DH],
                            src.rearrange("p (h d) -> p h d", h=8))
                        doff += cnt
                        r0 += cnt
                    vg.append((t, n))

                # ---- attention per head ----
                pvs = []
                dns = []
                for h in range(H):
                    ci, off = h // 2, (h % 2) * 64
                    pchunks = []
                    for ki, (s, n) in enumerate(KCHUNKS):
                        psc = ps.tile([128, ROWS], F32, tag="ps")
                        nc.tensor.matmul(psc[0:n, :],
                                         kg[ci][off:off + 64, s:s + n],
                                         qT[ci][off:off + 64, :],
                                         start=True, stop=True)
                        pe = atp.tile([128, ROWS], F32R, tag="pch")
                        nc.scalar.activation(pe[0:n, :], psc[0:n, :], AF.Exp,
                                             scale=1.0 / math.sqrt(DH))
                        if s == NC:  # buffer columns: causal/visibility mask
                            nc.vector.tensor_mul(pe[0:n, :], pe[0:n, :], t_bufm[:])
                        pchunks.append((pe, n))
                    pavt = pav.tile([DH + 1, ROWS], F32, tag="pav")
                    for ki, ((vt, n), (pe, n2)) in enumerate(zip(vg, pchunks)):
                        nc.tensor.matmul(pavt[:], vt[0:n, h, :], pe[0:n, :],
                                         start=(ki == 0), stop=(ki == len(vg) - 1))
                    pv = pvp.tile([128, ROWS], F32, tag=f"pv{h}")
                    nc.vector.tensor_copy(pv[(h % 2) * 64:(h % 2) * 64 + 64, :],
                                          pavt[0:DH, :])
                    dn = pvp.tile([1, ROWS], F32R, tag=f"dn{h}")
                    nc.vector.tensor_copy(dn[:], pavt[DH:DH + 1, :])
                    pvs.append(pv)
                    dns.append(dn)

                # denominators: assemble [8, ROWS] in psum, add self term
                pden = pt.tile([15, ROWS], F32, tag="pt")
                for h in range(H):
                    nc.tensor.matmul(pden[0:8, :], t_sel8[:, h * 8:(h + 1) * 8],
                                     dns[h][:], start=(h == 0), stop=False)
                nc.tensor.matmul(pden[0:8, :], t_eye8[:], pself[:],
                                 start=False, stop=True)
                rec = smp.tile([8, ROWS], F32R, tag="rec")
                nc.vector.reciprocal(rec[:], pden[0:8, :])
                selfw = smp.tile([8, ROWS], F32R, tag="selfw")
                nc.vector.tensor_mul(selfw[:], pself[:], rec[:])

                attn = []
                for ci in range(4):
                    prb = ps.tile([128, ROWS], F32, tag="ps")
                    nc.tensor.matmul(prb[:], t_exp[:, ci * 128:(ci + 1) * 128],
                                     rec[:], start=True, stop=True)
                    psw = ps.tile([128, ROWS], F32, tag="ps")
                    nc.tensor.matmul(psw[:], t_exp[:, ci * 128:(ci + 1) * 128],
                                     selfw[:], start=True, stop=True)
                    t = actp.tile([128, ROWS], F32R, tag=f"attn{ci}")
                    nc.vector.tensor_mul(t[:], vT[ci][:], psw[:])
                    t2 = actp.tile([128, ROWS], F32, tag="t2")
                    for hh in range(2):
                        o = hh * 64
                        nc.vector.tensor_mul(t2[o:o + 64, :],
                                             pvs[2 * ci + hh][o:o + 64, :],
                                             prb[o:o + 64, :])
                        nc.vector.tensor_add(t[o:o + 64, :],
                                             t[o:o + 64, :], t2[o:o + 64, :])
                    attn.append(t)

                # ---- o-proj + residual + LN2 ----
                w_o = load_w(lw["wo"], D, wdp, "wd")
                b_o = load_b(lw["bo"], "bo")
                g2 = load_b(lw["g2"], "g2"); c2 = load_b(lw["c2"], "c2")

                def epi_o(psum, co):
                    t = actp.tile([128, ROWS], F32R, tag=f"res{co}")
                    nc.scalar.activation(t[:], psum[:], AF.Identity, bias=b_o[co][:])
                    nc.vector.tensor_add(t[:], t[:], xf[co][:])
                    return t
                r2 = proj_t(attn, w_o, epi_o, D)
                x2 = layernorm(r2, g2, c2, "x2")

                # ---- MLP + residual + LN3 ----
                w_f1 = load_w(lw["wf1"], DFF, wffp, "wf")
                b_f1 = load_b(lw["bf1"], "bf1", DFF)

                def epi_g(psum, co):
                    t = actp.tile([128, ROWS], F32R, tag=f"h1_{co}")
                    nc.scalar.activation(t[:], psum[:], AF.Gelu, bias=b_f1[co][:])
                    return t
                h1 = proj_t(x2, w_f1, epi_g, DFF)

                w_f2 = load_w(lw["wf2"], D, wffp, "wf")
                b_f2 = load_b(lw["bf2"], "bf2")
                g3 = load_b(lw["g3"], "g3"); c3 = load_b(lw["c3"], "c3")

                def epi_f2(psum, co):
                    t = actp.tile([128, ROWS], F32R, tag=f"res{co}")
                    nc.scalar.activation(t[:], psum[:], AF.Identity, bias=b_f2[co][:])
                    nc.vector.tensor_add(t[:], t[:], x2[co][:])
                    return t
                r3 = proj_t(h1, w_f2, epi_f2, D)
                x = layernorm(r3, g3, c3, "x")

            # ---- final norm + head ----
            gft = load_b(gf, "gf"); cft = load_b(cf, "cf")
            z = layernorm(x, gft, cft, "res")
            w_h1 = load_w(wh1, DFF, wffp, "wf")
            b_h1 = load_b(bh1, "bh1", DFF)

            def epi_h1(psum, co):
                t = actp.tile([128, ROWS], F32R, tag=f"h1_{co}")
                nc.scalar.activation(t[:], psum[:], AF.Gelu, bias=b_h1[co][:])
                return t
            hh = proj_t(z, w_h1, epi_h1, DFF)

            w_h2 = load_w(wh2, 15, wffp, "wf")
            b_h2t = smp.tile([15, 1], F32, tag="bh2")
            nc.sync.dma_start(b_h2t[:], bh2[:])
            ph2 = pt.tile([15, ROWS], F32, tag="pt")
            for ci in range(8):
                nc.tensor.matmul(ph2[:], w_h2[ci][:], hh[ci][:],
                                 start=(ci == 0), stop=(ci == 7))
            rawt = smp.tile([15, ROWS], F32, tag="rawt")
            nc.scalar.activation(rawt[:], ph2[:], AF.Identity, bias=b_h2t[:])
            nc.sync.dma_start(raw_out[:], rawt[:])

    nc.compile()
    return nc


def _prep_inputs(params, x_context, y_context, x_buffer, y_buffer,
                 x_target, y_target):
    g = lambda a: np.asarray(a, dtype=np.float32)

    # ---- host embedding (tiny: ~1.6M FLOPs) ----
    wx, bx = g(params["x_embed"]["W"]), g(params["x_embed"]["b"])
    wy, by = g(params["y_embed"]["W"]), g(params["y_embed"]["b"])
    marker = g(params["marker"])
    ar = g(params["ar_tokens"])
    def emb(xx, yy=None):
        e = g(xx).mean(axis=2, keepdims=True) @ wx + bx
        if yy is not None:
            e = e + g(yy)[..., None] @ wy + by
        return e
    ctx = emb(x_context, y_context) + marker[1]
    buf = emb(x_buffer, y_buffer) + marker[2] + ar
    tgt = emb(x_target) + marker[0]
    x = np.concatenate([ctx, buf, tgt], axis=1)          # [B, R, D]

    # ---- per-layer fused weights ----
    layers = []
    for lp in params["layers"]:
        wvf, bvf = g(lp["attn_f"]["v"]["W"]), g(lp["attn_f"]["v"]["b"])
        wof, bof = g(lp["attn_f"]["o"]["W"]), g(lp["attn_f"]["o"]["b"])
        wo, bo = g(lp["attn_r"]["o"]["W"]), g(lp["attn_r"]["o"]["b"])
        bv = g(lp["attn_r"]["v"]["b"])
        layers.append({
            "wfvo": wvf @ wof, "bfvo": (bvf @ wof + bof)[:, None],
            "wq": g(lp["attn_r"]["q"]["W"]), "bq": g(lp["attn_r"]["q"]["b"])[:, None],
            "wk": g(lp["attn_r"]["k"]["W"]),
            "wv": g(lp["attn_r"]["v"]["W"]),
            "wo": wo, "bo": (bv @ wo + bo)[:, None],
            "wf1": g(lp["ff1"]["W"]), "bf1": g(lp["ff1"]["b"])[:, None],
            "wf2": g(lp["ff2"]["W"]), "bf2": g(lp["ff2"]["b"])[:, None],
            "g1": g(lp["n1"]["g"])[:, None], "c1": g(lp["n1"]["b"])[:, None],
            "g2": g(lp["n2"]["g"])[:, None], "c2": g(lp["n2"]["b"])[:, None],
            "g3": g(lp["n3"]["g"])[:, None], "c3": g(lp["n3"]["b"])[:, None],
        })

    # ---- masks / constants ----
    idx = np.arange(R)
    is_tgt = (idx >= NKV).astype(np.float32)
    # buffer-column visibility (cols NC..NC+NB) per query row
    bm = np.zeros((R, NB), np.float32)
    bcol = NC + np.arange(NB)
    is_buf_row = (idx >= NC) & (idx < NKV)
    bm[is_buf_row[:, None] & (bcol[None, :] <= idx[:, None])] = 1.0
    bm[idx >= NKV, :] = 1.0

    expander = np.zeros((8, D), np.float32)
    for m in range(D):
        expander[m // DH, m] = 1.0
    blockones = np.zeros((128, 32), np.float32)
    for ci in range(4):
        blockones[0:64, ci * 8 + 2 * ci] = 1.0
        blockones[64:128, ci * 8 + 2 * ci + 1] = 1.0

    sel8 = np.zeros((1, 64), np.float32)
    for h in range(8):
        sel8[0, h * 8 + h] = 1.0
    common = {
        "sel8": sel8,
        "eye8": np.eye(8, dtype=np.float32),
        "vones": np.ones((128, 8), np.float32),
        "cones": np.ones((128, 1), np.float32),
        "onesrow": np.ones((1, 128), np.float32),
        "blockones": blockones,
        "expander": expander,
        "wh1": g(params["head1"]["W"]), "bh1": g(params["head1"]["b"])[:, None],
        "wh2": g(params["head2"]["W"]), "bh2": g(params["head2"]["b"])[:, None],
        "gf": g(params["final_norm"]["g"])[:, None],
        "cf": g(params["final_norm"]["b"])[:, None],
    }
    for l, lw in enumerate(layers):
        for k, v in lw.items():
            name = f"{k}{l}" if not k[-1].isdigit() else f"{k}_{l}"
            common[name] = np.ascontiguousarray(v)

    # bf16 for everything except per-partition bias/scale vectors (f32 in graph)
    f32_names = {"bfvo", "bq", "bo", "bf1", "bf2", "g1", "c1", "g2", "c2",
                 "g3", "c3", "bh", "bh1", "bh2", "gf", "cf"}
    def is_f32(name):
        base = name.rstrip("0123456789").rstrip("_")
        return base in f32_names
    common = {k: (v if is_f32(k) else v.astype(ml_dtypes.bfloat16))
              for k, v in common.items()}
    in_maps = []
    for core in range(N_CORES):
        b, p = core // 4, core % 4
        r0, r1 = p * ROWS, (p + 1) * ROWS
        m = dict(common)
        m["x0"] = np.ascontiguousarray(x[b, r0:r1, :].T).astype(ml_dtypes.bfloat16)
        m["bufmask"] = np.ascontiguousarray(bm[r0:r1, :].T).astype(ml_dtypes.bfloat16)
        m["istgt8"] = np.ascontiguousarray(
            np.repeat(is_tgt[None, r0:r1], NB, axis=0)).astype(ml_dtypes.bfloat16)
        in_maps.append(m)
    return in_maps


def _epilogue(raws, params, y_target):
    """raws: [B, R, 15] head outputs; mixture + loss in numpy."""
    g = lambda a: np.asarray(a, dtype=np.float32)
    h = raws[:, NKV:, :]                                  # [B, NT, 15]
    raw = h.reshape(B, NT, K, 1, 3)
    mean = raw[..., 0] + g(params["mean_bias"])[None, None, :, None]
    sp_in = raw[..., 1] + g(params["std_bias"])[None, None, :, None]
    std = np.minimum(np.logaddexp(0.0, sp_in), 2.0) + STD_MIN
    wl = raw[..., 2] + g(params["weight_bias"])[None, None, :, None]
    wmax = wl.max(axis=2, keepdims=True)
    we = np.exp(wl - wmax)
    w = we / we.sum(axis=2, keepdims=True)
    yt = g(y_target)[:, :, None, None]
    logp = (-0.5 * (math.log(2 * math.pi) + 2 * np.log(std)
                    + ((yt - mean) / std) ** 2)
            + np.log(np.clip(w, 1e-12, None)))
    lmax = logp.max(axis=2, keepdims=True)
    ll = np.log(np.exp(logp - lmax).sum(axis=2)) + lmax[:, :, 0, :]
    loss = -ll.mean()
    return (np.float32(loss), mean.astype(np.float32),
            std.astype(np.float32), w.astype(np.float32))


def kernel(params, x_context, y_context, x_buffer, y_buffer,
           x_target, y_target, mask_features, mask_rows):
    global _COMPILED
    if _COMPILED is None:
        _COMPILED = _build()
    in_maps = _prep_inputs(params, x_context, y_context, x_buffer, y_buffer,
                           x_target, y_target)
    res = run_bass_kernel_spmd(_COMPILED, in_maps, core_ids=list(range(N_CORES)))
    raws = np.zeros((B, R, 15), np.float32)
    for core in range(N_CORES):
        b, p = core // 4, core % 4
        raws[b, p * ROWS:(p + 1) * ROWS, :] = res.results[core]["raw"].T
    return _epilogue(raws, params, y_target)


# revision 19
# speedup vs baseline: 1.6323x; 1.0985x over previous
"""ARTabPFN forward kernel for 8 TRN2 NeuronCores.

Sharding: 2 batch groups x 4-way row sharding (386 rows/core).
Device does: 4 transformer layers + final norm + head MLP.
Host does: embedding (tiny) and the K=5 mixture/loss epilogue (tiny).

Activations live transposed in SBUF: [feature_on_partitions, rows_on_free].
All matmuls run as float32r (full PE speed, ~1e-4 rounding).
Row attention exploits sparsity: every row attends only to the first
NC+NB=1032 columns (ctx+buf), plus a masked self-term for target rows,
so K/V are gathered per layer only for those 1032 rows (AllGather over
each 4-core group).
"""

import math
import numpy as np
import ml_dtypes

import concourse.bass as bass
import concourse.bacc as bacc
import concourse.tile as tile
import concourse.mybir as mybir
from concourse.bass_utils import run_bass_kernel_spmd

F32R = mybir.dt.bfloat16  # compute dtype (bf16: FWL fast weight load, half DMA/AG bytes)
F32 = mybir.dt.float32
AF = mybir.ActivationFunctionType

B, NC, NB, NT, CF = 2, 1024, 8, 512, 32
D, H, L, DFF, K = 512, 8, 4, 1024, 5
R = NC + NB + NT          # 1544
NKV = NC + NB             # 1032
ROWS = R // 4             # 386 rows per core
DH = D // H               # 64
N_CORES = 8
STD_MIN = 1e-3

# peer p in a group owns rows [p*ROWS, (p+1)*ROWS); kv rows are < NKV
PEER_KV = [min(max(NKV - p * ROWS, 0), ROWS) for p in range(4)]   # [386,386,260,0]
CONTRIB_ROWS = D + ROWS   # legacy (unused)
KCHUNKS = [(s, min(128, NKV - s)) for s in range(0, NKV, 128)]    # 9 chunks, last = 8

_COMPILED = None


def _build():
    nc = bacc.Bacc("TRN2", target_bir_lowering=False, debug=False,
                   num_devices=N_CORES)

    def din(name, shape, dt=F32R):
        return nc.declare_dram_parameter(name, list(shape), dt, isOutput=False)

    x0 = din("x0", [D, ROWS])
    cones = din("cones", [128, 1])
    onesrow = din("onesrow", [1, 128])
    blockones = din("blockones", [128, 32])
    expander = din("expander", [8, D])
    sel8 = din("sel8", [1, 64])
    eye8 = din("eye8", [8, 8])
    vones = din("vones", [128, 8])
    bufmask = din("bufmask", [NB, ROWS])
    istgt8 = din("istgt8", [NB, ROWS])

    LW = []
    for l in range(L):
        LW.append({
            "wfvo": din(f"wfvo{l}", [D, D]), "bfvo": din(f"bfvo{l}", [D, 1], F32),
            "wq": din(f"wq{l}", [D, D]), "bq": din(f"bq{l}", [D, 1], F32),
            "wk": din(f"wk{l}", [D, D]),
            "wv": din(f"wv{l}", [D, D]),
            "wo": din(f"wo{l}", [D, D]), "bo": din(f"bo{l}", [D, 1], F32),
            "wf1": din(f"wf1_{l}", [D, DFF]), "bf1": din(f"bf1_{l}", [DFF, 1], F32),
            "wf2": din(f"wf2_{l}", [DFF, D]), "bf2": din(f"bf2_{l}", [D, 1], F32),
            "g1": din(f"g1_{l}", [D, 1], F32), "c1": din(f"c1_{l}", [D, 1], F32),
            "g2": din(f"g2_{l}", [D, 1], F32), "c2": din(f"c2_{l}", [D, 1], F32),
            "g3": din(f"g3_{l}", [D, 1], F32), "c3": din(f"c3_{l}", [D, 1], F32),
        })
    wh1 = din("wh1", [D, DFF]); bh1 = din("bh1", [DFF, 1], F32)
    wh2 = din("wh2", [DFF, 15]); bh2 = din("bh2", [15, 1], F32)
    gf = din("gf", [D, 1], F32); cf = din("cf", [D, 1], F32)

    raw_out = nc.declare_dram_parameter("raw", [15, ROWS], F32, isOutput=True)

    with tile.TileContext(nc) as tc:
        ctx_lp = nc.allow_low_precision(reason="deliberate f32r compute")
        ctx_lp.__enter__()
        with tc.tile_pool(name="const", bufs=1) as constp, \
             tc.tile_pool(name="acts", bufs=1) as actp, \
             tc.tile_pool(name="wd", bufs=2) as wdp, \
             tc.tile_pool(name="wff", bufs=1) as wffp, \
             tc.tile_pool(name="wvp", bufs=1) as wvp, \
             tc.tile_pool(name="kv", bufs=1) as kvp, \
             tc.tile_pool(name="attn", bufs=27) as atp, \
             tc.tile_pool(name="pvp", bufs=1) as pvp, \
             tc.tile_pool(name="small", bufs=1) as smp, \
             tc.tile_pool(name="pp", bufs=2, space="PSUM") as pp, \
             tc.tile_pool(name="ps", bufs=2, space="PSUM") as ps, \
             tc.tile_pool(name="pav", bufs=2, space="PSUM") as pav, \
             tc.tile_pool(name="pt", bufs=2, space="PSUM") as pt, \
             tc.tile_pool(name="dram", bufs=2, space="DRAM") as dramp:

            # ---- constants ----
            t_cones = constp.tile([128, 1], F32R, tag="cones")
            nc.sync.dma_start(t_cones[:], cones[:])
            t_onesrow = constp.tile([1, 128], F32R, tag="onesrow")
            nc.sync.dma_start(t_onesrow[:], onesrow[:])
            t_blko = constp.tile([128, 32], F32R, tag="blko")
            nc.sync.dma_start(t_blko[:], blockones[:])
            t_exp = constp.tile([8, D], F32R, tag="exp")
            nc.sync.dma_start(t_exp[:], expander[:])
            t_sel8 = constp.tile([1, 64], F32R, tag="sel8")
            nc.sync.dma_start(t_sel8[:], sel8[:])
            t_eye8 = constp.tile([8, 8], F32R, tag="eye8")
            nc.sync.dma_start(t_eye8[:], eye8[:])
            t_bufm = constp.tile([NB, ROWS], F32R, tag="bufm")
            nc.sync.dma_start(t_bufm[:], bufmask[:])
            t_istgt = constp.tile([NB, ROWS], F32R, tag="istgt")
            nc.sync.dma_start(t_istgt[:], istgt8[:])

            def load_w(drh, dout, pool, tag):
                """Load [D_in, dout] weights as tiles of [128, dout]."""
                nin = drh.shape[0]
                ts = []
                for ci in range(nin // 128):
                    t = pool.tile([128, dout], F32R, tag=f"{tag}{ci}")
                    nc.sync.dma_start(t[:], drh[ci * 128:(ci + 1) * 128, :])
                    ts.append(t)
                return ts

            def load_b(drh, tag, n=D):
                ts = []
                for ci in range(n // 128):
                    t = smp.tile([128, 1], F32, tag=f"{tag}{ci}")
                    nc.sync.dma_start(t[:], drh[ci * 128:(ci + 1) * 128, :])
                    ts.append(t)
                return ts

            def proj_t(xin, wts, epi, dout):
                """Transposed projection: out[co][128, ROWS] = sum_ci W[ci][:,co].T @ xin[ci].
                epi(psum, co) -> sbuf tile."""
                outs = []
                for co in range(dout // 128):
                    psum = ps.tile([128, ROWS], F32, tag="ps")
                    nci = len(xin)
                    for ci in range(nci):
                        nc.tensor.matmul(psum[:], wts[ci][:, co * 128:(co + 1) * 128],
                                         xin[ci][:], start=(ci == 0), stop=(ci == nci - 1))
                    outs.append(epi(psum, co))
                return outs

            def layernorm(xin, gts, cts, tag):
                """LN over features (partitions). xin: 4 tiles [128, ROWS] f32r."""
                pst = pt.tile([15, ROWS], F32, tag="pt")
                for ci in range(4):
                    nc.tensor.matmul(pst[0:1, :], t_cones[:], xin[ci][:],
                                     start=(ci == 0), stop=(ci == 3))
                mu = smp.tile([1, ROWS], F32, tag="mu")
                nc.vector.tensor_scalar_mul(mu[:], pst[0:1, :], 1.0 / D)
                psq = pt.tile([15, ROWS], F32, tag="pt")
                for ci in range(4):
                    sq = actp.tile([128, ROWS], F32R, tag="lnsq")
                    nc.vector.tensor_mul(sq[:], xin[ci][:], xin[ci][:])
                    nc.tensor.matmul(psq[0:1, :], t_cones[:], sq[:],
                                     start=(ci == 0), stop=(ci == 3))
                var = smp.tile([1, ROWS], F32, tag="var")
                nc.vector.tensor_mul(var[:], mu[:], mu[:])
                nc.vector.tensor_scalar_sub(var[:], var[:], 0.0)
                exq = smp.tile([1, ROWS], F32, tag="exq")
                nc.vector.tensor_scalar_mul(exq[:], psq[0:1, :], 1.0 / D)
                nc.vector.tensor_sub(var[:], exq[:], var[:])
                nc.vector.tensor_scalar_add(var[:], var[:], 1e-5)
                nc.scalar.activation(exq[:], var[:], AF.Sqrt)
                rr = smp.tile([1, ROWS], F32R, tag="rr")
                nc.vector.reciprocal(rr[:], exq[:])
                aa = smp.tile([1, ROWS], F32R, tag="aa")
                nc.vector.tensor_mul(aa[:], mu[:], rr[:])
                nc.vector.tensor_scalar_mul(aa[:], aa[:], -1.0)
                # broadcast r, a across partitions via K=1 matmul
                pbs = ps.tile([128, ROWS], F32, tag="ps")
                nc.tensor.matmul(pbs[:], t_onesrow[:], rr[:], start=True, stop=True)
                pba = ps.tile([128, ROWS], F32, tag="ps")
                nc.tensor.matmul(pba[:], t_onesrow[:], aa[:], start=True, stop=True)
                outs = []
                for ci in range(4):
                    t = actp.tile([128, ROWS], F32, tag="lnt")
                    nc.vector.tensor_mul(t[:], xin[ci][:], pbs[:])
                    nc.vector.tensor_add(t[:], t[:], pba[:])
                    o = actp.tile([128, ROWS], F32R, tag=f"{tag}{ci}")
                    nc.scalar.activation(o[:], t[:], AF.Identity,
                                         bias=cts[ci][:], scale=gts[ci][:])
                    outs.append(o)
                return outs

            # ---- initial activations ----
            x = []
            for ci in range(4):
                t = actp.tile([128, ROWS], F32R, tag=f"x{ci}")
                nc.sync.dma_start(t[:], x0[ci * 128:(ci + 1) * 128, :])
                x.append(t)

            for l in range(L):
                lw = LW[l]
                w_fvo = load_w(lw["wfvo"], D, wdp, "wd")
                b_fvo = load_b(lw["bfvo"], "bfvo")
                g1 = load_b(lw["g1"], "g1"); c1 = load_b(lw["c1"], "c1")

                # ---- feature attention (collapsed) + residual + LN1 ----
                def epi_fattn(psum, co):
                    t = actp.tile([128, ROWS], F32R, tag=f"res{co}")
                    nc.scalar.activation(t[:], psum[:], AF.Identity, bias=b_fvo[co][:])
                    nc.vector.tensor_add(t[:], t[:], x[co][:])
                    return t
                r1 = proj_t(x, w_fvo, epi_fattn, D)
                xf = layernorm(r1, g1, c1, "xf")

                # ---- K (no bias: cancels in softmax) and V row-major (bias folded into o-proj) ----
                w_k = load_w(lw["wk"], D, wdp, "wd")

                def epi_plain(tag):
                    def f(psum, co):
                        t = actp.tile([128, ROWS], F32R, tag=f"{tag}{co}")
                        nc.vector.tensor_copy(t[:], psum[:])
                        return t
                    return f
                w_v = load_w(lw["wv"], D, wvp, "wv")
                contribk = dramp.tile([D, ROWS], F32R, tag="contribk")
                gatheredk = dramp.tile([4 * D, ROWS], F32R, tag="gatheredk")
                contribv = dramp.tile([ROWS, D], F32R, tag="contribv")
                gatheredv = dramp.tile([4 * ROWS, D], F32R, tag="gatheredv")

                def epi_k(psum, co):
                    t = actp.tile([128, ROWS], F32R, tag=f"kT{co}")
                    nc.vector.tensor_copy(t[:], psum[:])
                    nc.sync.dma_start(contribk[co * 128:(co + 1) * 128, :], t[:])
                    return t
                kT = proj_t(xf, w_k, epi_k, D)
                nc.gpsimd.collective_compute(
                    "AllGather", mybir.AluOpType.bypass,
                    ins=[contribk.opt()], outs=[gatheredk.opt()],
                    replica_groups=[[0, 1, 2, 3], [4, 5, 6, 7]])
                # V row-major: out[row_chunk, 512]
                vrm = []
                for rt in range(4):
                    rc = min(128, ROWS - rt * 128)
                    psum = pp.tile([128, D], F32, tag="pp")
                    for ci in range(4):
                        nc.tensor.matmul(psum[0:rc, :],
                                         xf[ci][:, rt * 128:rt * 128 + rc],
                                         w_v[ci][:], start=(ci == 0), stop=(ci == 3))
                    t = actp.tile([128, D], F32R, tag=f"vrm{rt}")
                    nc.vector.tensor_copy(t[0:rc, :], psum[0:rc, :])
                    vrm.append((t, rc))
                    nc.sync.dma_start(contribv[rt * 128:rt * 128 + rc, :], t[0:rc, :])
                nc.gpsimd.collective_compute(
                    "AllGather", mybir.AluOpType.bypass,
                    ins=[contribv.opt()], outs=[gatheredv.opt()],
                    replica_groups=[[0, 1, 2, 3], [4, 5, 6, 7]])

                # ---- Q (with bias), vT (transposed V for self term) while gather flies ----
                w_q = load_w(lw["wq"], D, wdp, "wd")
                b_q = load_b(lw["bq"], "bq")

                def epi_q(psum, co):
                    t = actp.tile([128, ROWS], F32R, tag=f"qT{co}")
                    nc.scalar.activation(t[:], psum[:], AF.Identity, bias=b_q[co][:])
                    return t
                qT = proj_t(xf, w_q, epi_q, D)
                vT = proj_t(xf, w_v, epi_plain("vT"), D)

                # self scores: s_self[h, row] = sum_dh qT*kT
                pss = pt.tile([15, ROWS], F32, tag="pt")
                for ci in range(4):
                    t = actp.tile([128, ROWS], F32R, tag="qk")
                    nc.vector.tensor_mul(t[:], qT[ci][:], kT[ci][:])
                    nc.tensor.matmul(pss[0:8, :], t_blko[:, ci * 8:(ci + 1) * 8],
                                     t[:], start=(ci == 0), stop=(ci == 3))
                sst = smp.tile([15, ROWS], F32, tag="rawt")
                nc.vector.tensor_copy(sst[0:8, :], pss[0:8, :])
                pself = smp.tile([8, ROWS], F32R, tag="pself")
                nc.scalar.activation(pself[:], sst[0:8, :], AF.Exp, scale=1.0 / math.sqrt(DH))
                nc.vector.tensor_mul(pself[:], pself[:], t_istgt[:])

                # ---- repack gathered K/V ----
                kg = []
                for ci in range(4):
                    t = kvp.tile([128, NKV], F32R, tag=f"kg{ci}")
                    off = 0
                    for p in range(4):
                        n = PEER_KV[p]
                        if n == 0:
                            continue
                        nc.sync.dma_start(
                            t[:, off:off + n],
                            gatheredk[p * D + ci * 128:p * D + (ci + 1) * 128, 0:n])
                        off += n
                    kg.append(t)
                vg = []
                for (s, n) in KCHUNKS:
                    t = kvp.tile([128, 8, DH + 1], F32R, tag=f"vg{s}")
                    nc.sync.dma_start(t[0:n, :, DH:DH + 1], vones[0:n, :])
                    # rows s..s+n map onto peer blocks
                    r0 = s
                    doff = 0
                    while r0 < s + n:
                        p = min(r0 // ROWS, 2)
                        lr0 = r0 - p * ROWS
                        cnt = min(PEER_KV[p] - lr0, s + n - r0)
                        src = gatheredv[p * ROWS + lr0:p * ROWS + lr0 + cnt, :]
                        nc.sync.dma_start(
                            t[doff:doff + cnt, :, 0:DH],
                            src.rearrange("p (h d) -> p h d", h=8))
                        doff += cnt
                        r0 += cnt
                    vg.append((t, n))

                # ---- attention: scores run 2 heads ahead of AV ----
                pvs = [None] * H
                dns = [None] * H
                all_p = [None] * H

                def do_scores(h):
                    ci, off = h // 2, (h % 2) * 64
                    pchunks = []
                    for ki, (s, n) in enumerate(KCHUNKS):
                        psc = ps.tile([128, ROWS], F32, tag="ps")
                        nc.tensor.matmul(psc[0:n, :],
                                         kg[ci][off:off + 64, s:s + n],
                                         qT[ci][off:off + 64, :],
                                         start=True, stop=True)
                        pe = atp.tile([128, ROWS], F32R, tag="pch")
                        nc.scalar.activation(pe[0:n, :], psc[0:n, :], AF.Exp,
                                             scale=1.0 / math.sqrt(DH))
                        if s == NC:  # buffer columns: causal/visibility mask
                            nc.vector.tensor_mul(pe[0:n, :], pe[0:n, :], t_bufm[:])
                        pchunks.append((pe, n))
                    all_p[h] = pchunks

                def do_av(h):
                    pchunks = all_p[h]
                    pavt = pav.tile([DH + 1, ROWS], F32, tag="pav")
                    for ki, ((vt, n), (pe, n2)) in enumerate(zip(vg, pchunks)):
                        nc.tensor.matmul(pavt[:], vt[0:n, h, :], pe[0:n, :],
                                         start=(ki == 0), stop=(ki == len(vg) - 1))
                    pv = pvp.tile([128, ROWS], F32, tag=f"pv{h}")
                    nc.vector.tensor_copy(pv[(h % 2) * 64:(h % 2) * 64 + 64, :],
                                          pavt[0:DH, :])
                    dn = pvp.tile([1, ROWS], F32R, tag=f"dn{h}")
                    nc.vector.tensor_copy(dn[:], pavt[DH:DH + 1, :])
                    pvs[h] = pv
                    dns[h] = dn
                    all_p[h] = None

                for h in range(H):
                    do_scores(h)
                    if h >= 2:
                        do_av(h - 2)
                do_av(H - 2)
                do_av(H - 1)

                # denominators: assemble [8, ROWS] in psum, add self term
                pden = pt.tile([15, ROWS], F32, tag="pt")
                for h in range(H):
                    nc.tensor.matmul(pden[0:8, :], t_sel8[:, h * 8:(h + 1) * 8],
                                     dns[h][:], start=(h == 0), stop=False)
                nc.tensor.matmul(pden[0:8, :], t_eye8[:], pself[:],
                                 start=False, stop=True)
                rec = smp.tile([8, ROWS], F32R, tag="rec")
                nc.vector.reciprocal(rec[:], pden[0:8, :])
                selfw = smp.tile([8, ROWS], F32R, tag="selfw")
                nc.vector.tensor_mul(selfw[:], pself[:], rec[:])

                attn = []
                for ci in range(4):
                    prb = ps.tile([128, ROWS], F32, tag="ps")
                    nc.tensor.matmul(prb[:], t_exp[:, ci * 128:(ci + 1) * 128],
                                     rec[:], start=True, stop=True)
                    psw = ps.tile([128, ROWS], F32, tag="ps")
                    nc.tensor.matmul(psw[:], t_exp[:, ci * 128:(ci + 1) * 128],
                                     selfw[:], start=True, stop=True)
                    t = actp.tile([128, ROWS], F32R, tag=f"attn{ci}")
                    nc.vector.tensor_mul(t[:], vT[ci][:], psw[:])
                    t2 = actp.tile([128, ROWS], F32, tag="t2")
                    for hh in range(2):
                        o = hh * 64
                        nc.vector.tensor_mul(t2[o:o + 64, :],
                                             pvs[2 * ci + hh][o:o + 64, :],
                                             prb[o:o + 64, :])
                        nc.vector.tensor_add(t[o:o + 64, :],
                                             t[o:o + 64, :], t2[o:o + 64, :])
                    attn.append(t)

                # ---- o-proj + residual + LN2 ----
                w_o = load_w(lw["wo"], D, wdp, "wd")
                b_o = load_b(lw["bo"], "bo")
                g2 = load_b(lw["g2"], "g2"); c2 = load_b(lw["c2"], "c2")

                def epi_o(psum, co):
                    t = actp.tile([128, ROWS], F32R, tag=f"res{co}")
                    nc.scalar.activation(t[:], psum[:], AF.Identity, bias=b_o[co][:])
                    nc.vector.tensor_add(t[:], t[:], xf[co][:])
                    return t
                r2 = proj_t(attn, w_o, epi_o, D)
                x2 = layernorm(r2, g2, c2, "x2")

                # ---- MLP + residual + LN3 ----
                w_f1 = load_w(lw["wf1"], DFF, wffp, "wf")
                b_f1 = load_b(lw["bf1"], "bf1", DFF)

                def epi_g(psum, co):
                    t = actp.tile([128, ROWS], F32R, tag=f"h1_{co}")
                    nc.scalar.activation(t[:], psum[:], AF.Gelu, bias=b_f1[co][:])
                    return t
                h1 = proj_t(x2, w_f1, epi_g, DFF)

                w_f2 = load_w(lw["wf2"], D, wffp, "wf")
                b_f2 = load_b(lw["bf2"], "bf2")
                g3 = load_b(lw["g3"], "g3"); c3 = load_b(lw["c3"], "c3")

                def epi_f2(psum, co):
                    t = actp.tile([128, ROWS], F32R, tag=f"res{co}")
                    nc.scalar.activation(t[:], psum[:], AF.Identity, bias=b_f2[co][:])
                    nc.vector.tensor_add(t[:], t[:], x2[co][:])
                    return t
                r3 = proj_t(h1, w_f2, epi_f2, D)
                x = layernorm(r3, g3, c3, "x")

            # ---- final norm + head ----
            gft = load_b(gf, "gf"); cft = load_b(cf, "cf")
            z = layernorm(x, gft, cft, "res")
            w_h1 = load_w(wh1, DFF, wffp, "wf")
            b_h1 = load_b(bh1, "bh1", DFF)

            def epi_h1(psum, co):
                t = actp.tile([128, ROWS], F32R, tag=f"h1_{co}")
                nc.scalar.activation(t[:], psum[:], AF.Gelu, bias=b_h1[co][:])
                return t
            hh = proj_t(z, w_h1, epi_h1, DFF)

            w_h2 = load_w(wh2, 15, wffp, "wf")
            b_h2t = smp.tile([15, 1], F32, tag="bh2")
            nc.sync.dma_start(b_h2t[:], bh2[:])
            ph2 = pt.tile([15, ROWS], F32, tag="pt")
            for ci in range(8):
                nc.tensor.matmul(ph2[:], w_h2[ci][:], hh[ci][:],
                                 start=(ci == 0), stop=(ci == 7))
            rawt = smp.tile([15, ROWS], F32, tag="rawt")
            nc.scalar.activation(rawt[:], ph2[:], AF.Identity, bias=b_h2t[:])
            nc.sync.dma_start(raw_out[:], rawt[:])

    nc.compile()
    return nc


def _prep_inputs(params, x_context, y_context, x_buffer, y_buffer,
                 x_target, y_target):
    g = lambda a: np.asarray(a, dtype=np.float32)

    # ---- host embedding (tiny: ~1.6M FLOPs) ----
    wx, bx = g(params["x_embed"]["W"]), g(params["x_embed"]["b"])
    wy, by = g(params["y_embed"]["W"]), g(params["y_embed"]["b"])
    marker = g(params["marker"])
    ar = g(params["ar_tokens"])
    def emb(xx, yy=None):
        e = g(xx).mean(axis=2, keepdims=True) @ wx + bx
        if yy is not None:
            e = e + g(yy)[..., None] @ wy + by
        return e
    ctx = emb(x_context, y_context) + marker[1]
    buf = emb(x_buffer, y_buffer) + marker[2] + ar
    tgt = emb(x_target) + marker[0]
    x = np.concatenate([ctx, buf, tgt], axis=1)          # [B, R, D]

    # ---- per-layer fused weights ----
    layers = []
    for lp in params["layers"]:
        wvf, bvf = g(lp["attn_f"]["v"]["W"]), g(lp["attn_f"]["v"]["b"])
        wof, bof = g(lp["attn_f"]["o"]["W"]), g(lp["attn_f"]["o"]["b"])
        wo, bo = g(lp["attn_r"]["o"]["W"]), g(lp["attn_r"]["o"]["b"])
        bv = g(lp["attn_r"]["v"]["b"])
        layers.append({
            "wfvo": wvf @ wof, "bfvo": (bvf @ wof + bof)[:, None],
            "wq": g(lp["attn_r"]["q"]["W"]), "bq": g(lp["attn_r"]["q"]["b"])[:, None],
            "wk": g(lp["attn_r"]["k"]["W"]),
            "wv": g(lp["attn_r"]["v"]["W"]),
            "wo": wo, "bo": (bv @ wo + bo)[:, None],
            "wf1": g(lp["ff1"]["W"]), "bf1": g(lp["ff1"]["b"])[:, None],
            "wf2": g(lp["ff2"]["W"]), "bf2": g(lp["ff2"]["b"])[:, None],
            "g1": g(lp["n1"]["g"])[:, None], "c1": g(lp["n1"]["b"])[:, None],
            "g2": g(lp["n2"]["g"])[:, None], "c2": g(lp["n2"]["b"])[:, None],
            "g3": g(lp["n3"]["g"])[:, None], "c3": g(lp["n3"]["b"])[:, None],
        })

    # ---- masks / constants ----
    idx = np.arange(R)
    is_tgt = (idx >= NKV).astype(np.float32)
    # buffer-column visibility (cols NC..NC+NB) per query row
    bm = np.zeros((R, NB), np.float32)
    bcol = NC + np.arange(NB)
    is_buf_row = (idx >= NC) & (idx < NKV)
    bm[is_buf_row[:, None] & (bcol[None, :] <= idx[:, None])] = 1.0
    bm[idx >= NKV, :] = 1.0

    expander = np.zeros((8, D), np.float32)
    for m in range(D):
        expander[m // DH, m] = 1.0
    blockones = np.zeros((128, 32), np.float32)
    for ci in range(4):
        blockones[0:64, ci * 8 + 2 * ci] = 1.0
        blockones[64:128, ci * 8 + 2 * ci + 1] = 1.0

    sel8 = np.zeros((1, 64), np.float32)
    for h in range(8):
        sel8[0, h * 8 + h] = 1.0
    common = {
        "sel8": sel8,
        "eye8": np.eye(8, dtype=np.float32),
        "vones": np.ones((128, 8), np.float32),
        "cones": np.ones((128, 1), np.float32),
        "onesrow": np.ones((1, 128), np.float32),
        "blockones": blockones,
        "expander": expander,
        "wh1": g(params["head1"]["W"]), "bh1": g(params["head1"]["b"])[:, None],
        "wh2": g(params["head2"]["W"]), "bh2": g(params["head2"]["b"])[:, None],
        "gf": g(params["final_norm"]["g"])[:, None],
        "cf": g(params["final_norm"]["b"])[:, None],
    }
    for l, lw in enumerate(layers):
        for k, v in lw.items():
            name = f"{k}{l}" if not k[-1].isdigit() else f"{k}_{l}"
            common[name] = np.ascontiguousarray(v)

    # bf16 for everything except per-partition bias/scale vectors (f32 in graph)
    f32_names = {"bfvo", "bq", "bo", "bf1", "bf2", "g1", "c1", "g2", "c2",
                 "g3", "c3", "bh", "bh1", "bh2", "gf", "cf"}
    def is_f32(name):
        base = name.rstrip("0123456789").rstrip("_")
        return base in f32_names
    common = {k: (v if is_f32(k) else v.astype(ml_dtypes.bfloat16))
              for k, v in common.items()}
    in_maps = []
    for core in range(N_CORES):
        b, p = core // 4, core % 4
        r0, r1 = p * ROWS, (p + 1) * ROWS
        m = dict(common)
        m["x0"] = np.ascontiguousarray(x[b, r0:r1, :].T).astype(ml_dtypes.bfloat16)
        m["bufmask"] = np.ascontiguousarray(bm[r0:r1, :].T).astype(ml_dtypes.bfloat16)
        m["istgt8"] = np.ascontiguousarray(
            np.repeat(is_tgt[None, r0:r1], NB, axis=0)).astype(ml_dtypes.bfloat16)
        in_maps.append(m)
    return in_maps


def _epilogue(raws, params, y_target):
    """raws: [B, R, 15] head outputs; mixture + loss in numpy."""
    g = lambda a: np.asarray(a, dtype=np.float32)
    h = raws[:, NKV:, :]                                  # [B, NT, 15]
    raw = h.reshape(B, NT, K, 1, 3)
    mean = raw[..., 0] + g(params["mean_bias"])[None, None, :, None]
    sp_in = raw[..., 1] + g(params["std_bias"])[None, None, :, None]
    std = np.minimum(np.logaddexp(0.0, sp_in), 2.0) + STD_MIN
    wl = raw[..., 2] + g(params["weight_bias"])[None, None, :, None]
    wmax = wl.max(axis=2, keepdims=True)
    we = np.exp(wl - wmax)
    w = we / we.sum(axis=2, keepdims=True)
    yt = g(y_target)[:, :, None, None]
    logp = (-0.5 * (math.log(2 * math.pi) + 2 * np.log(std)
                    + ((yt - mean) / std) ** 2)
            + np.log(np.clip(w, 1e-12, None)))
    lmax = logp.max(axis=2, keepdims=True)
    ll = np.log(np.exp(logp - lmax).sum(axis=2)) + lmax[:, :, 0, :]
    loss = -ll.mean()
    return (np.float32(loss), mean.astype(np.float32),
            std.astype(np.float32), w.astype(np.float32))


def kernel(params, x_context, y_context, x_buffer, y_buffer,
           x_target, y_target, mask_features, mask_rows):
    global _COMPILED
    if _COMPILED is None:
        _COMPILED = _build()
    in_maps = _prep_inputs(params, x_context, y_context, x_buffer, y_buffer,
                           x_target, y_target)
    res = run_bass_kernel_spmd(_COMPILED, in_maps, core_ids=list(range(N_CORES)))
    raws = np.zeros((B, R, 15), np.float32)
    for core in range(N_CORES):
        b, p = core // 4, core % 4
        raws[b, p * ROWS:(p + 1) * ROWS, :] = res.results[core]["raw"].T
    return _epilogue(raws, params, y_target)
